# revision 22
# baseline (speedup 1.0000x reference)
"""Trainium2 Bass kernel for nn_Attention_7945689497706.

Distribution: data-parallel over batch, 2 batch elements per core, weights
replicated, no collectives.

Per-core layout:
  - RMSNorm via ones-matmul partition reduction, gamma folded into weights.
  - q^T,k^T in [o, n] fp32r; v in [n, o] feeding a bf16 [v|1] (j, 65) tile.
  - Attention transposed (j on psum partitions): sim_T = kTpad^T qT with K
    zero-padded to 128; exp on ACT at [128,1024] grain; av lhsT = vext so the
    ones column accumulates softmax denominators; normalization = K=1 matmul
    broadcast + DVE fast-reciprocal + multiply.
  - mem_kv + padding in a 9th j-chunk (zero k-cols / zero v-rows make the
    padded lanes contribute nothing).
  - The two batch elements are software-pipelined: batch 1's norm/projections
    are emitted inside batch 0's attention loop (per-head kTp handoff) so the
    PE fills the ACT-bound exp bubbles.
"""

import numpy as np

import concourse.bass as bass
import concourse.mybir as mybir
import concourse.tile as tile
from concourse import bacc
from concourse.bass_utils import run_bass_kernel_spmd

F32 = mybir.dt.float32
F32R = mybir.dt.float32r
BF16 = mybir.dt.bfloat16
AF = mybir.ActivationFunctionType

NCORES = 8
B = 16
C = 512
N = 1024          # pixels = 32*32
HEADS = 8
DH = 64
NMEM = 4
PB = B // NCORES  # batch elements per core
CT = C // 128     # channel partition-tiles
JC = 9            # j chunks: 8 pixel chunks + 1 (mem + zero pad)
VW = HEADS * (DH + 1)  # vext width: per head [v | ones] = 65


def _build():
    nc = bacc.Bacc()
    x_ext = nc.declare_dram_parameter("x", [PB, C, N], F32, isOutput=False)
    wqkvt_ext = nc.declare_dram_parameter("wqkvt", [C, 3 * C], F32, isOutput=False)
    wot_ext = nc.declare_dram_parameter("wot", [C, C], F32, isOutput=False)
    gammat_ext = nc.declare_dram_parameter("gammat", [128, CT], F32, isOutput=False)
    memk_ext = nc.declare_dram_parameter("memk", [128, HEADS, 128], F32, isOutput=False)
    memv_ext = nc.declare_dram_parameter("memv", [128, VW], F32, isOutput=False)
    out_ext = nc.declare_dram_parameter("out", [PB, C, N], F32, isOutput=True)

    with tile.TileContext(nc) as tc:
        with (
            tc.tile_pool(name="const", bufs=1) as const,
            tc.tile_pool(name="wstage", bufs=1) as wstage,
            tc.tile_pool(name="xp", bufs=2) as xp,
            tc.tile_pool(name="data", bufs=1) as data,
            tc.tile_pool(name="qp", bufs=2) as qp,
            tc.tile_pool(name="pp", bufs=4) as pp,
            tc.tile_pool(name="avs", bufs=2) as avsp,
            tc.tile_pool(name="rp", bufs=2) as rp,
            tc.tile_pool(name="ob", bufs=2) as obp,
            tc.tile_pool(name="qkv_ps", bufs=2, space="PSUM") as qkv_ps,
            tc.tile_pool(name="sim_ps", bufs=2, space="PSUM") as sim_ps,
            tc.tile_pool(name="av_ps", bufs=2, space="PSUM") as av_ps,
        ):
            # ------------ batch-0 x load first (weights stream behind it) -------
            xraws = []
            for bb in range(PB):
                xr = xp.tile([128, CT, N], F32, tag="xraw")
                xraws.append(xr)
            for t in range(CT):
                nc.sync.dma_start(out=xraws[0][:, t, :], in_=x_ext[0, t * 128:(t + 1) * 128, :])

            # ---------------- per-core constants ----------------
            wqkv = const.tile([128, CT, 3 * C], BF16, tag="wqkv")
            wo = const.tile([128, CT, C], BF16, tag="wo")
            g1 = const.tile([128, CT], F32, tag="g1")
            g1q = const.tile([128, CT], F32, tag="g1q")
            ones128 = const.tile([128, 128], BF16, tag="ones128")
            ones1 = const.tile([128, 64], F32R, tag="ones1")
            kTp = const.tile([128, HEADS, 128 * JC], BF16, tag="kTp")
            vextA = const.tile([128, JC, VW], BF16, tag="vextA")
            vextB = const.tile([128, JC, VW], BF16, tag="vextB")
            vexts = [vextA, vextB]

            gsb = const.tile([128, CT], F32, tag="gsb")
            nc.sync.dma_start(out=gsb, in_=gammat_ext[:, :])
            nc.scalar.activation(out=g1, in_=gsb, func=AF.Copy, bias=1.0)
            nc.scalar.activation(out=g1q, in_=gsb, func=AF.Copy, bias=1.0, scale=1.0)
            nc.scalar.mul(out=g1q, in_=g1q, mul=DH ** -0.5)

            nc.vector.memset(ones128, 1.0)
            nc.vector.memset(ones1.bitcast(F32), 1.0)

            def weight_prep():
                nc.gpsimd.memset(kTp, 0.0)
                for t in range(CT):
                    ws = wstage.tile([128, 3 * C], F32, tag="ws")
                    nc.scalar.dma_start(out=ws, in_=wqkvt_ext[t * 128:(t + 1) * 128, :])
                    nc.vector.tensor_scalar_mul(
                        out=wqkv[:, t, 0:C], in0=ws[:, 0:C], scalar1=g1q[:, t:t + 1])
                    nc.vector.tensor_scalar_mul(
                        out=wqkv[:, t, C:3 * C], in0=ws[:, C:3 * C], scalar1=g1[:, t:t + 1])
                for t in range(CT):
                    ws = wstage.tile([128, 3 * C], F32, tag="ws")
                    nc.scalar.dma_start(out=ws[:, 0:C], in_=wot_ext[t * 128:(t + 1) * 128, :])
                    nc.vector.tensor_copy(out=wo[:, t, :], in_=ws[:, 0:C])
                # mem_kv constants -> bf16 tiles (9th j-chunk)
                ws = wstage.tile([128, 3 * C], F32, tag="ws")
                nc.sync.dma_start(out=ws[:, 0:HEADS * 128],
                                  in_=memk_ext[:, :, :].rearrange("p h c -> p (h c)"))
                nc.vector.tensor_copy(
                    out=kTp[:, :, 8 * 128:9 * 128],
                    in_=ws[:, 0:HEADS * 128].rearrange("p (h c) -> p h c", c=128))
                ws2 = wstage.tile([128, 3 * C], F32, tag="ws")
                nc.sync.dma_start(out=ws2[:, 0:VW], in_=memv_ext[:, :])
                for v in vexts:
                    nc.gpsimd.memset(v, 0.0)
                    nc.vector.tensor_copy(out=v[:, 8, :], in_=ws2[:, 0:VW])
                    oc = v[:, 0:8, :].rearrange("p j (h c) -> p j h c", c=DH + 1)[:, :, :, DH:DH + 1]
                    nc.gpsimd.memset(oc, 1.0)

            # ---------------- pipeline stages ----------------
            def norm(bb):
                """x -> xn (fp32r, per-pixel normalized)."""
                xraw = xraws[bb]
                xsq = data.tile([128, CT, N], BF16, tag="xsq")
                for t in range(CT):
                    nc.vector.tensor_mul(out=xsq[:, t, :], in0=xraw[:, t, :], in1=xraw[:, t, :])
                ss = sim_ps.tile([128, N], F32, tag="sim")
                for h2 in range(2):
                    for t in range(CT):
                        nc.tensor.matmul(ss[:, h2 * 512:(h2 + 1) * 512], ones128,
                                         xsq[:, t, h2 * 512:(h2 + 1) * 512],
                                         start=(t == 0), stop=(t == CT - 1))
                sroot = data.tile([128, N], F32, tag="sroot")
                nc.scalar.activation(out=sroot, in_=ss, func=AF.Sqrt, scale=1.0 / C)
                snorm = data.tile([128, N], F32, tag="snorm")
                nc.vector.reciprocal_approx_fast(out=snorm, in_=sroot)
                xn = data.tile([128, CT, N], BF16, tag="xn" + str(bb))
                for t in range(CT):
                    nc.vector.tensor_mul(out=xn[:, t, :], in0=xraw[:, t, :], in1=snorm)
                return xn

            def qkproj(xn, qT, mcs):
                """o-chunks mcs of the q/k projection; k goes into kTp (padded)."""
                for mc in mcs:
                    for h2 in range(2):
                        ps = qkv_ps.tile([128, 512], F32, tag="q")
                        for t in range(CT):
                            nc.tensor.matmul(ps, wqkv[:, t, mc * 128:(mc + 1) * 128],
                                             xn[:, t, h2 * 512:(h2 + 1) * 512],
                                             start=(t == 0), stop=(t == CT - 1))
                        if mc < 4:
                            nc.vector.tensor_copy(out=qT[:, mc, h2 * 512:(h2 + 1) * 512], in_=ps)
                        else:
                            h0, h1 = 2 * (mc - 4), 2 * (mc - 4) + 1
                            nc.vector.tensor_copy(
                                out=kTp[0:64, h0, h2 * 512:(h2 + 1) * 512], in_=ps[0:64, :])
                            nc.vector.tensor_copy(
                                out=kTp[64:128, h1, h2 * 512:(h2 + 1) * 512], in_=ps[64:128, :])

            def vproj(xn, vext, ics):
                for ic in ics:
                    ps = qkv_ps.tile([128, 512], F32, tag="q")
                    for t in range(CT):
                        nc.tensor.matmul(ps, xn[:, t, ic * 128:(ic + 1) * 128],
                                         wqkv[:, t, 2 * C:3 * C],
                                         start=(t == 0), stop=(t == CT - 1))
                    ps_h = ps[:, :].rearrange("p (h c) -> p h c", c=DH)
                    vdst = vext[:, ic, :].rearrange("p (h c) -> p h c", c=DH + 1)[:, :, 0:DH]
                    nc.vector.tensor_copy(out=vdst, in_=ps_h)

            def head_attn(h, qT, vext, attn):
                av0 = av_ps.tile([65, 512], F32, tag="av")
                av1 = av_ps.tile([65, 512], F32, tag="av")
                avt = (av0, av1)
                for jc in range(JC):
                    st = sim_ps.tile([128, N], F32, tag="sim")
                    for h2 in range(2):
                        nc.tensor.matmul(st[:, h2 * 512:(h2 + 1) * 512],
                                         kTp[:, h, jc * 128:(jc + 1) * 128],
                                         qT[:, h // 2, h2 * 512:(h2 + 1) * 512],
                                         start=True, stop=True)
                    p = pp.tile([128, N], BF16, tag="p")
                    nc.scalar.activation(out=p, in_=st, func=AF.Exp)
                    for h2 in range(2):
                        nc.tensor.matmul(avt[h2], vext[:, jc, h * (DH + 1):(h + 1) * (DH + 1)],
                                         p[:, h2 * 512:(h2 + 1) * 512],
                                         start=(jc == 0), stop=(jc == JC - 1))
                for h2 in range(2):
                    avb = avsp.tile([65, 512], F32R, tag="avs")
                    nc.vector.tensor_copy(out=avb, in_=avt[h2])
                    bc = qkv_ps.tile([64, 512], F32, tag="q")
                    nc.tensor.matmul(bc, ones1[64:65, :], avb[64:65, :], start=True, stop=True)
                    rcp = rp.tile([64, 512], F32, tag="rcp")
                    nc.vector.reciprocal_approx_fast(out=rcp, in_=bc)
                    nc.vector.tensor_mul(
                        out=attn[64 * (h % 2):64 * (h % 2) + 64, h // 2,
                                 h2 * 512:(h2 + 1) * 512],
                        in0=avb[0:64, :].bitcast(F32), in1=rcp)

            def proj(attn, bb):
                for mc in range(CT):
                    for h2 in range(2):
                        ps = qkv_ps.tile([128, 512], F32, tag="q")
                        for t in range(CT):
                            nc.tensor.matmul(ps, wo[:, t, mc * 128:(mc + 1) * 128],
                                             attn[:, t, h2 * 512:(h2 + 1) * 512],
                                             start=(t == 0), stop=(t == CT - 1))
                        ob = obp.tile([128, 512], F32, tag="ob")
                        nc.vector.tensor_copy(out=ob, in_=ps)
                        nc.sync.dma_start(
                            out=out_ext[bb, mc * 128:(mc + 1) * 128, h2 * 512:(h2 + 1) * 512],
                            in_=ob)

            # ---------------- interleaved schedule ----------------
            xn0 = norm(0)
            weight_prep()
            for t in range(CT):
                nc.sync.dma_start(out=xraws[1][:, t, :], in_=x_ext[1, t * 128:(t + 1) * 128, :])
            qT0 = qp.tile([128, CT, N], BF16, tag="qT")
            qkproj(xn0, qT0, range(8))
            vproj(xn0, vexts[0], range(8))
            xn1 = norm(1)

            qT1 = qp.tile([128, CT, N], BF16, tag="qT")
            attn0 = data.tile([128, CT, N], BF16, tag="attn")
            for h in range(HEADS):
                head_attn(h, qT0, vexts[0], attn0)
                # batch 1 projections fill the exp-bound bubbles; k chunks are
                # written into kTp right after batch 0 finishes reading them.
                qkproj(xn1, qT1, [h // 2] if h % 2 == 0 else [4 + (h - 1) // 2])
                vproj(xn1, vexts[1], [h])
            proj(attn0, 0)

            attn1 = data.tile([128, CT, N], BF16, tag="attn")
            for h in range(HEADS):
                head_attn(h, qT1, vexts[1], attn1)
            proj(attn1, 1)
    nc.compile()
    return nc


_NC_CACHE = []


def kernel(x, gamma, mem_kv, w_qkv, w_out, _trace=False):
    x = np.asarray(x, dtype=np.float32)
    gamma = np.asarray(gamma, dtype=np.float32)
    mem_kv = np.asarray(mem_kv, dtype=np.float32)
    w_qkv = np.asarray(w_qkv, dtype=np.float32)
    w_out = np.asarray(w_out, dtype=np.float32)

    b, c, hh, ww = x.shape
    n = hh * ww
    xs = x.reshape(b, c, n)

    wqkvt = np.ascontiguousarray(w_qkv.T)          # [c, 3c]
    wot = np.ascontiguousarray(w_out.T)            # [c, c]
    gammat = np.ascontiguousarray(gamma.reshape(CT, 128).T)  # [128, CT]

    memk = np.zeros((128, HEADS, 128), np.float32)
    memv = np.zeros((128, VW), np.float32)
    for h in range(HEADS):
        r0 = 64 * (h % 2)
        memk[r0:r0 + DH, h, 0:NMEM] = mem_kv[0, h].T      # [dh, nmem]
        memv[0:NMEM, h * (DH + 1):h * (DH + 1) + DH] = mem_kv[1, h]
        memv[0:NMEM, h * (DH + 1) + DH] = 1.0

    if not _NC_CACHE:
        _NC_CACHE.append(_build())
    nc = _NC_CACHE[0]

    in_maps = []
    for core in range(NCORES):
        in_maps.append({
            "x": np.ascontiguousarray(xs[core * PB:(core + 1) * PB]),
            "wqkvt": wqkvt,
            "wot": wot,
            "gammat": gammat,
            "memk": memk,
            "memv": memv,
        })
    res = run_bass_kernel_spmd(nc, in_maps, core_ids=list(range(NCORES)), trace=_trace)
    out = np.concatenate([res.results[core]["out"] for core in range(NCORES)], axis=0)
    kernel.last_result = res
    return out.reshape(b, c, hh, ww)


# revision 23
# speedup vs baseline: 1.0712x; 1.0712x over previous
"""Trainium2 Bass kernel for nn_Attention_7945689497706.

Distribution: data-parallel over batch, 2 batch elements per core, weights
replicated, no collectives.

Per-core layout:
  - RMSNorm via ones-matmul partition reduction, gamma folded into weights.
  - q^T,k^T in [o, n] fp32r; v in [n, o] feeding a bf16 [v|1] (j, 65) tile.
  - Attention transposed (j on psum partitions): sim_T = kTpad^T qT with K
    zero-padded to 128; exp on ACT at [128,1024] grain; av lhsT = vext so the
    ones column accumulates softmax denominators; normalization = K=1 matmul
    broadcast + DVE fast-reciprocal + multiply.
  - mem_kv + padding in a 9th j-chunk (zero k-cols / zero v-rows make the
    padded lanes contribute nothing).
  - The two batch elements are software-pipelined: batch 1's norm/projections
    are emitted inside batch 0's attention loop (per-head kTp handoff) so the
    PE fills the ACT-bound exp bubbles.
"""

import numpy as np

import concourse.bass as bass
import concourse.mybir as mybir
import concourse.tile as tile
from concourse import bacc
from concourse.bass_utils import run_bass_kernel_spmd

F32 = mybir.dt.float32
F32R = mybir.dt.float32r
BF16 = mybir.dt.bfloat16
AF = mybir.ActivationFunctionType

NCORES = 8
B = 16
C = 512
N = 1024          # pixels = 32*32
HEADS = 8
DH = 64
NMEM = 4
PB = B // NCORES  # batch elements per core
CT = C // 128     # channel partition-tiles
JC = 9            # j chunks: 8 pixel chunks + 1 (mem + zero pad)
VW = HEADS * (DH + 1)  # vext width: per head [v | ones] = 65


def _build():
    nc = bacc.Bacc()
    x_ext = nc.declare_dram_parameter("x", [PB, C, N], F32, isOutput=False)
    wqkvt_ext = nc.declare_dram_parameter("wqkvt", [C, 3 * C], F32, isOutput=False)
    wot_ext = nc.declare_dram_parameter("wot", [C, C], F32, isOutput=False)
    gammat_ext = nc.declare_dram_parameter("gammat", [128, CT], F32, isOutput=False)
    memk_ext = nc.declare_dram_parameter("memk", [128, HEADS, 128], F32, isOutput=False)
    memv_ext = nc.declare_dram_parameter("memv", [128, VW], F32, isOutput=False)
    out_ext = nc.declare_dram_parameter("out", [PB, C, N], F32, isOutput=True)

    with tile.TileContext(nc) as tc:
        with (
            tc.tile_pool(name="const", bufs=1) as const,
            tc.tile_pool(name="wstage", bufs=1) as wstage,
            tc.tile_pool(name="xp", bufs=2) as xp,
            tc.tile_pool(name="data", bufs=1) as data,
            tc.tile_pool(name="qp", bufs=2) as qp,
            tc.tile_pool(name="pp", bufs=4) as pp,
            tc.tile_pool(name="avs", bufs=2) as avsp,
            tc.tile_pool(name="rp", bufs=2) as rp,
            tc.tile_pool(name="ob", bufs=2) as obp,
            tc.tile_pool(name="qkv_ps", bufs=2, space="PSUM") as qkv_ps,
            tc.tile_pool(name="sim_ps", bufs=2, space="PSUM") as sim_ps,
            tc.tile_pool(name="av_ps", bufs=2, space="PSUM") as av_ps,
        ):
            # ------------ batch-0 x load first (weights stream behind it) -------
            xraws = []
            for bb in range(PB):
                xr = xp.tile([128, CT, N], F32, tag="xraw")
                xraws.append(xr)
            for t in range(CT):
                nc.sync.dma_start(out=xraws[0][:, t, :], in_=x_ext[0, t * 128:(t + 1) * 128, :])

            # ---------------- per-core constants ----------------
            wqkv = const.tile([128, CT, 3 * C], BF16, tag="wqkv")
            wo = const.tile([128, CT, C], BF16, tag="wo")
            g1 = const.tile([128, CT], F32, tag="g1")
            g1q = const.tile([128, CT], F32, tag="g1q")
            ones128 = const.tile([128, 128], BF16, tag="ones128")
            ones1 = const.tile([128, 64], F32R, tag="ones1")
            kTp = const.tile([128, HEADS, 128 * JC], BF16, tag="kTp")
            vextA = const.tile([128, JC, VW], BF16, tag="vextA")
            vextB = const.tile([128, JC, VW], BF16, tag="vextB")
            vexts = [vextA, vextB]

            gsb = const.tile([128, CT], F32, tag="gsb")
            nc.sync.dma_start(out=gsb, in_=gammat_ext[:, :])
            nc.scalar.activation(out=g1, in_=gsb, func=AF.Copy, bias=1.0)
            nc.scalar.activation(out=g1q, in_=gsb, func=AF.Copy, bias=1.0, scale=1.0)
            nc.scalar.mul(out=g1q, in_=g1q, mul=DH ** -0.5)

            nc.vector.memset(ones128, 1.0)
            nc.vector.memset(ones1.bitcast(F32), 1.0)

            def weight_prep():
                nc.gpsimd.memset(kTp, 0.0)
                for t in range(CT):
                    ws = wstage.tile([128, 3 * C], F32, tag="ws")
                    nc.scalar.dma_start(out=ws, in_=wqkvt_ext[t * 128:(t + 1) * 128, :])
                    nc.vector.tensor_scalar_mul(
                        out=wqkv[:, t, 0:C], in0=ws[:, 0:C], scalar1=g1q[:, t:t + 1])
                    nc.vector.tensor_scalar_mul(
                        out=wqkv[:, t, C:3 * C], in0=ws[:, C:3 * C], scalar1=g1[:, t:t + 1])
                for t in range(CT):
                    ws = wstage.tile([128, 3 * C], F32, tag="ws")
                    nc.scalar.dma_start(out=ws[:, 0:C], in_=wot_ext[t * 128:(t + 1) * 128, :])
                    nc.vector.tensor_copy(out=wo[:, t, :], in_=ws[:, 0:C])
                # mem_kv constants -> bf16 tiles (9th j-chunk)
                ws = wstage.tile([128, 3 * C], F32, tag="ws")
                nc.sync.dma_start(out=ws[:, 0:HEADS * 128],
                                  in_=memk_ext[:, :, :].rearrange("p h c -> p (h c)"))
                nc.vector.tensor_copy(
                    out=kTp[:, :, 8 * 128:9 * 128],
                    in_=ws[:, 0:HEADS * 128].rearrange("p (h c) -> p h c", c=128))
                ws2 = wstage.tile([128, 3 * C], F32, tag="ws")
                nc.sync.dma_start(out=ws2[:, 0:VW], in_=memv_ext[:, :])
                for v in vexts:
                    nc.gpsimd.memset(v, 0.0)
                    nc.vector.tensor_copy(out=v[:, 8, :], in_=ws2[:, 0:VW])
                    oc = v[:, 0:8, :].rearrange("p j (h c) -> p j h c", c=DH + 1)[:, :, :, DH:DH + 1]
                    nc.gpsimd.memset(oc, 1.0)

            # ---------------- pipeline stages ----------------
            def norm(bb):
                """x -> xn (fp32r, per-pixel normalized)."""
                xraw = xraws[bb]
                xsq = data.tile([128, CT, N], BF16, tag="xsq")
                for t in range(CT):
                    nc.vector.tensor_mul(out=xsq[:, t, :], in0=xraw[:, t, :], in1=xraw[:, t, :])
                ss = sim_ps.tile([128, N], F32, tag="sim")
                for h2 in range(2):
                    for t in range(CT):
                        nc.tensor.matmul(ss[:, h2 * 512:(h2 + 1) * 512], ones128,
                                         xsq[:, t, h2 * 512:(h2 + 1) * 512],
                                         start=(t == 0), stop=(t == CT - 1))
                sroot = data.tile([128, N], F32, tag="sroot")
                nc.scalar.activation(out=sroot, in_=ss, func=AF.Sqrt, scale=1.0 / C)
                snorm = data.tile([128, N], F32, tag="snorm")
                nc.vector.reciprocal_approx_fast(out=snorm, in_=sroot)
                xn = data.tile([128, CT, N], BF16, tag="xn" + str(bb))
                for t in range(CT):
                    nc.vector.tensor_mul(out=xn[:, t, :], in0=xraw[:, t, :], in1=snorm)
                return xn

            def qkproj(xn, qT, mcs):
                """o-chunks mcs of the q/k projection; k goes into kTp (padded)."""
                for mc in mcs:
                    for h2 in range(2):
                        ps = qkv_ps.tile([128, 512], F32, tag="q")
                        for t in range(CT):
                            nc.tensor.matmul(ps, wqkv[:, t, mc * 128:(mc + 1) * 128],
                                             xn[:, t, h2 * 512:(h2 + 1) * 512],
                                             start=(t == 0), stop=(t == CT - 1))
                        if mc < 4:
                            nc.vector.tensor_copy(out=qT[:, mc, h2 * 512:(h2 + 1) * 512], in_=ps)
                        else:
                            h0, h1 = 2 * (mc - 4), 2 * (mc - 4) + 1
                            nc.vector.tensor_copy(
                                out=kTp[0:64, h0, h2 * 512:(h2 + 1) * 512], in_=ps[0:64, :])
                            nc.vector.tensor_copy(
                                out=kTp[64:128, h1, h2 * 512:(h2 + 1) * 512], in_=ps[64:128, :])

            def vproj(xn, vext, ics):
                for ic in ics:
                    ps = qkv_ps.tile([128, 512], F32, tag="q")
                    for t in range(CT):
                        nc.tensor.matmul(ps, xn[:, t, ic * 128:(ic + 1) * 128],
                                         wqkv[:, t, 2 * C:3 * C],
                                         start=(t == 0), stop=(t == CT - 1))
                    ps_h = ps[:, :].rearrange("p (h c) -> p h c", c=DH)
                    vdst = vext[:, ic, :].rearrange("p (h c) -> p h c", c=DH + 1)[:, :, 0:DH]
                    nc.vector.tensor_copy(out=vdst, in_=ps_h)

            def head_attn(h, qT, vext, attn):
                av0 = av_ps.tile([65, 512], F32, tag="av")
                av1 = av_ps.tile([65, 512], F32, tag="av")
                avt = (av0, av1)
                for jc in range(JC):
                    st = sim_ps.tile([128, N], F32, tag="sim")
                    for h2 in range(2):
                        nc.tensor.matmul(st[:, h2 * 512:(h2 + 1) * 512],
                                         kTp[:, h, jc * 128:(jc + 1) * 128],
                                         qT[:, h // 2, h2 * 512:(h2 + 1) * 512],
                                         start=True, stop=True)
                    p = pp.tile([128, N], BF16, tag="p")
                    nc.scalar.activation(out=p, in_=st, func=AF.Exp)
                    for h2 in range(2):
                        nc.tensor.matmul(avt[h2], vext[:, jc, h * (DH + 1):(h + 1) * (DH + 1)],
                                         p[:, h2 * 512:(h2 + 1) * 512],
                                         start=(jc == 0), stop=(jc == JC - 1))
                for h2 in range(2):
                    avb = avsp.tile([65, 512], F32R, tag="avs")
                    nc.vector.tensor_copy(out=avb, in_=avt[h2])
                    bc = av_ps.tile([64, 512], F32, tag="av")
                    nc.tensor.matmul(bc, ones1[64:65, :], avb[64:65, :], start=True, stop=True)
                    rcp = rp.tile([64, 512], F32, tag="rcp")
                    nc.vector.reciprocal_approx_fast(out=rcp, in_=bc)
                    nc.vector.tensor_mul(
                        out=attn[64 * (h % 2):64 * (h % 2) + 64, h // 2,
                                 h2 * 512:(h2 + 1) * 512],
                        in0=avb[0:64, :].bitcast(F32), in1=rcp)

            def proj(attn, bb):
                for mc in range(CT):
                    for h2 in range(2):
                        ps = qkv_ps.tile([128, 512], F32, tag="q")
                        for t in range(CT):
                            nc.tensor.matmul(ps, wo[:, t, mc * 128:(mc + 1) * 128],
                                             attn[:, t, h2 * 512:(h2 + 1) * 512],
                                             start=(t == 0), stop=(t == CT - 1))
                        ob = obp.tile([128, 512], F32, tag="ob")
                        nc.vector.tensor_copy(out=ob, in_=ps)
                        nc.sync.dma_start(
                            out=out_ext[bb, mc * 128:(mc + 1) * 128, h2 * 512:(h2 + 1) * 512],
                            in_=ob)

            # ---------------- interleaved schedule ----------------
            xn0 = norm(0)
            weight_prep()
            for t in range(CT):
                nc.sync.dma_start(out=xraws[1][:, t, :], in_=x_ext[1, t * 128:(t + 1) * 128, :])
            qT0 = qp.tile([128, CT, N], BF16, tag="qT")
            qkproj(xn0, qT0, range(8))
            vproj(xn0, vexts[0], range(8))
            xn1 = norm(1)

            qT1 = qp.tile([128, CT, N], BF16, tag="qT")
            attn0 = data.tile([128, CT, N], BF16, tag="attn")
            for h in range(HEADS):
                head_attn(h, qT0, vexts[0], attn0)
                # batch 1 projections fill the exp-bound bubbles; k chunks are
                # written into kTp right after batch 0 finishes reading them.
                qkproj(xn1, qT1, [h // 2] if h % 2 == 0 else [4 + (h - 1) // 2])
                vproj(xn1, vexts[1], [h])
            proj(attn0, 0)

            attn1 = data.tile([128, CT, N], BF16, tag="attn")
            for h in range(HEADS):
                head_attn(h, qT1, vexts[1], attn1)
            proj(attn1, 1)
    nc.compile()
    return nc


_NC_CACHE = []


def kernel(x, gamma, mem_kv, w_qkv, w_out, _trace=False):
    x = np.asarray(x, dtype=np.float32)
    gamma = np.asarray(gamma, dtype=np.float32)
    mem_kv = np.asarray(mem_kv, dtype=np.float32)
    w_qkv = np.asarray(w_qkv, dtype=np.float32)
    w_out = np.asarray(w_out, dtype=np.float32)

    b, c, hh, ww = x.shape
    n = hh * ww
    xs = x.reshape(b, c, n)

    wqkvt = np.ascontiguousarray(w_qkv.T)          # [c, 3c]
    wot = np.ascontiguousarray(w_out.T)            # [c, c]
    gammat = np.ascontiguousarray(gamma.reshape(CT, 128).T)  # [128, CT]

    memk = np.zeros((128, HEADS, 128), np.float32)
    memv = np.zeros((128, VW), np.float32)
    for h in range(HEADS):
        r0 = 64 * (h % 2)
        memk[r0:r0 + DH, h, 0:NMEM] = mem_kv[0, h].T      # [dh, nmem]
        memv[0:NMEM, h * (DH + 1):h * (DH + 1) + DH] = mem_kv[1, h]
        memv[0:NMEM, h * (DH + 1) + DH] = 1.0

    if not _NC_CACHE:
        _NC_CACHE.append(_build())
    nc = _NC_CACHE[0]

    in_maps = []
    for core in range(NCORES):
        in_maps.append({
            "x": np.ascontiguousarray(xs[core * PB:(core + 1) * PB]),
            "wqkvt": wqkvt,
            "wot": wot,
            "gammat": gammat,
            "memk": memk,
            "memv": memv,
        })
    res = run_bass_kernel_spmd(nc, in_maps, core_ids=list(range(NCORES)), trace=_trace)
    out = np.concatenate([res.results[core]["out"] for core in range(NCORES)], axis=0)
    kernel.last_result = res
    return out.reshape(b, c, hh, ww)


# revision 24
# speedup vs baseline: 1.0819x; 1.0100x over previous
"""Trainium2 Bass kernel for nn_Attention_7945689497706.

Distribution: data-parallel over batch, 2 batch elements per core, weights
replicated, no collectives.

Per-core layout:
  - RMSNorm via ones-matmul partition reduction, gamma folded into weights.
  - q^T,k^T in [o, n] fp32r; v in [n, o] feeding a bf16 [v|1] (j, 65) tile.
  - Attention transposed (j on psum partitions): sim_T = kTpad^T qT with K
    zero-padded to 128; exp on ACT at [128,1024] grain; av lhsT = vext so the
    ones column accumulates softmax denominators; normalization = K=1 matmul
    broadcast + DVE fast-reciprocal + multiply.
  - mem_kv + padding in a 9th j-chunk (zero k-cols / zero v-rows make the
    padded lanes contribute nothing).
  - The two batch elements are software-pipelined: batch 1's norm/projections
    are emitted inside batch 0's attention loop (per-head kTp handoff) so the
    PE fills the ACT-bound exp bubbles.
"""

import numpy as np

import concourse.bass as bass
import concourse.mybir as mybir
import concourse.tile as tile
from concourse import bacc
from concourse.bass_utils import run_bass_kernel_spmd

F32 = mybir.dt.float32
F32R = mybir.dt.float32r
BF16 = mybir.dt.bfloat16
AF = mybir.ActivationFunctionType

NCORES = 8
B = 16
C = 512
N = 1024          # pixels = 32*32
HEADS = 8
DH = 64
NMEM = 4
PB = B // NCORES  # batch elements per core
CT = C // 128     # channel partition-tiles
JC = 9            # j chunks: 8 pixel chunks + 1 (mem + zero pad)
VW = HEADS * (DH + 1)  # vext width: per head [v | ones] = 65


def _build():
    nc = bacc.Bacc()
    x_ext = nc.declare_dram_parameter("x", [PB, C, N], F32, isOutput=False)
    wqkvt_ext = nc.declare_dram_parameter("wqkvt", [C, 3 * C], F32, isOutput=False)
    wot_ext = nc.declare_dram_parameter("wot", [C, C], F32, isOutput=False)
    gammat_ext = nc.declare_dram_parameter("gammat", [128, CT], F32, isOutput=False)
    memk_ext = nc.declare_dram_parameter("memk", [128, HEADS, 128], F32, isOutput=False)
    memv_ext = nc.declare_dram_parameter("memv", [128, VW], F32, isOutput=False)
    out_ext = nc.declare_dram_parameter("out", [PB, C, N], F32, isOutput=True)

    with tile.TileContext(nc) as tc:
        with (
            tc.tile_pool(name="const", bufs=1) as const,
            tc.tile_pool(name="wstage", bufs=1) as wstage,
            tc.tile_pool(name="xp", bufs=2) as xp,
            tc.tile_pool(name="data", bufs=1) as data,
            tc.tile_pool(name="qp", bufs=2) as qp,
            tc.tile_pool(name="pp", bufs=4) as pp,
            tc.tile_pool(name="avs", bufs=2) as avsp,
            tc.tile_pool(name="rp", bufs=2) as rp,
            tc.tile_pool(name="ob", bufs=2) as obp,
            tc.tile_pool(name="qkv_ps", bufs=2, space="PSUM") as qkv_ps,
            tc.tile_pool(name="sim_ps", bufs=2, space="PSUM") as sim_ps,
            tc.tile_pool(name="av_ps", bufs=2, space="PSUM") as av_ps,
        ):
            # ------------ batch-0 x load first (weights stream behind it) -------
            xraws = []
            for bb in range(PB):
                xr = xp.tile([128, CT, N], F32, tag="xraw")
                xraws.append(xr)
            for t in range(CT):
                nc.sync.dma_start(out=xraws[0][:, t, :], in_=x_ext[0, t * 128:(t + 1) * 128, :])

            # ---------------- per-core constants ----------------
            wqkv = const.tile([128, CT, 3 * C], BF16, tag="wqkv")
            wo = const.tile([128, CT, C], BF16, tag="wo")
            g1 = const.tile([128, CT], F32, tag="g1")
            g1q = const.tile([128, CT], F32, tag="g1q")
            ones128 = const.tile([128, 128], BF16, tag="ones128")
            ones1 = const.tile([128, 64], F32R, tag="ones1")
            kTp = const.tile([128, HEADS, 128 * JC], BF16, tag="kTp")
            vextA = const.tile([128, JC, VW], BF16, tag="vextA")
            vextB = const.tile([128, JC, VW], BF16, tag="vextB")
            vexts = [vextA, vextB]

            gsb = const.tile([128, CT], F32, tag="gsb")
            nc.sync.dma_start(out=gsb, in_=gammat_ext[:, :])
            nc.scalar.activation(out=g1, in_=gsb, func=AF.Copy, bias=1.0)
            nc.scalar.activation(out=g1q, in_=gsb, func=AF.Copy, bias=1.0, scale=1.0)
            nc.scalar.mul(out=g1q, in_=g1q, mul=DH ** -0.5)

            nc.vector.memset(ones128, 1.0)
            nc.vector.memset(ones1.bitcast(F32), 1.0)

            def weight_prep():
                nc.gpsimd.memset(kTp, 0.0)
                for t in range(CT):
                    ws = wstage.tile([128, 3 * C], F32, tag="ws")
                    nc.sync.dma_start(out=ws, in_=wqkvt_ext[t * 128:(t + 1) * 128, :])
                    nc.vector.tensor_scalar_mul(
                        out=wqkv[:, t, 0:C], in0=ws[:, 0:C], scalar1=g1q[:, t:t + 1])
                    nc.vector.tensor_scalar_mul(
                        out=wqkv[:, t, C:3 * C], in0=ws[:, C:3 * C], scalar1=g1[:, t:t + 1])
                for t in range(CT):
                    ws = wstage.tile([128, 3 * C], F32, tag="ws")
                    nc.sync.dma_start(out=ws[:, 0:C], in_=wot_ext[t * 128:(t + 1) * 128, :])
                    nc.vector.tensor_copy(out=wo[:, t, :], in_=ws[:, 0:C])
                # mem_kv constants -> bf16 tiles (9th j-chunk)
                ws = wstage.tile([128, 3 * C], F32, tag="ws")
                nc.sync.dma_start(out=ws[:, 0:HEADS * 128],
                                  in_=memk_ext[:, :, :].rearrange("p h c -> p (h c)"))
                nc.vector.tensor_copy(
                    out=kTp[:, :, 8 * 128:9 * 128],
                    in_=ws[:, 0:HEADS * 128].rearrange("p (h c) -> p h c", c=128))
                ws2 = wstage.tile([128, 3 * C], F32, tag="ws")
                nc.sync.dma_start(out=ws2[:, 0:VW], in_=memv_ext[:, :])
                for v in vexts:
                    nc.gpsimd.memset(v, 0.0)
                    nc.vector.tensor_copy(out=v[:, 8, :], in_=ws2[:, 0:VW])
                    oc = v[:, 0:8, :].rearrange("p j (h c) -> p j h c", c=DH + 1)[:, :, :, DH:DH + 1]
                    nc.gpsimd.memset(oc, 1.0)

            # ---------------- pipeline stages ----------------
            def norm(bb):
                """x -> xn (fp32r, per-pixel normalized)."""
                xraw = xraws[bb]
                xsq = data.tile([128, CT, N], BF16, tag="xsq")
                for t in range(CT):
                    nc.vector.tensor_mul(out=xsq[:, t, :], in0=xraw[:, t, :], in1=xraw[:, t, :])
                ss = sim_ps.tile([128, N], F32, tag="sim")
                for h2 in range(2):
                    for t in range(CT):
                        nc.tensor.matmul(ss[:, h2 * 512:(h2 + 1) * 512], ones128,
                                         xsq[:, t, h2 * 512:(h2 + 1) * 512],
                                         start=(t == 0), stop=(t == CT - 1))
                sroot = data.tile([128, N], F32, tag="sroot")
                nc.scalar.activation(out=sroot, in_=ss, func=AF.Sqrt, scale=1.0 / C)
                snorm = data.tile([128, N], F32, tag="snorm")
                nc.vector.reciprocal_approx_fast(out=snorm, in_=sroot)
                xn = data.tile([128, CT, N], BF16, tag="xn" + str(bb))
                for t in range(CT):
                    nc.vector.tensor_mul(out=xn[:, t, :], in0=xraw[:, t, :], in1=snorm)
                return xn

            def qkproj(xn, qT, mcs):
                """o-chunks mcs of the q/k projection; k goes into kTp (padded)."""
                for mc in mcs:
                    for h2 in range(2):
                        ps = qkv_ps.tile([128, 512], F32, tag="q")
                        for t in range(CT):
                            nc.tensor.matmul(ps, wqkv[:, t, mc * 128:(mc + 1) * 128],
                                             xn[:, t, h2 * 512:(h2 + 1) * 512],
                                             start=(t == 0), stop=(t == CT - 1))
                        if mc < 4:
                            nc.vector.tensor_copy(out=qT[:, mc, h2 * 512:(h2 + 1) * 512], in_=ps)
                        else:
                            h0, h1 = 2 * (mc - 4), 2 * (mc - 4) + 1
                            nc.vector.tensor_copy(
                                out=kTp[0:64, h0, h2 * 512:(h2 + 1) * 512], in_=ps[0:64, :])
                            nc.vector.tensor_copy(
                                out=kTp[64:128, h1, h2 * 512:(h2 + 1) * 512], in_=ps[64:128, :])

            def vproj(xn, vext, ics):
                for ic in ics:
                    ps = qkv_ps.tile([128, 512], F32, tag="q")
                    for t in range(CT):
                        nc.tensor.matmul(ps, xn[:, t, ic * 128:(ic + 1) * 128],
                                         wqkv[:, t, 2 * C:3 * C],
                                         start=(t == 0), stop=(t == CT - 1))
                    ps_h = ps[:, :].rearrange("p (h c) -> p h c", c=DH)
                    vdst = vext[:, ic, :].rearrange("p (h c) -> p h c", c=DH + 1)[:, :, 0:DH]
                    nc.vector.tensor_copy(out=vdst, in_=ps_h)

            def head_attn(h, qT, vext, attn):
                av0 = av_ps.tile([65, 512], F32, tag="av")
                av1 = av_ps.tile([65, 512], F32, tag="av")
                avt = (av0, av1)
                for jc in range(JC):
                    st = sim_ps.tile([128, N], F32, tag="sim")
                    for h2 in range(2):
                        nc.tensor.matmul(st[:, h2 * 512:(h2 + 1) * 512],
                                         kTp[:, h, jc * 128:(jc + 1) * 128],
                                         qT[:, h // 2, h2 * 512:(h2 + 1) * 512],
                                         start=True, stop=True)
                    p = pp.tile([128, N], BF16, tag="p")
                    nc.scalar.activation(out=p, in_=st, func=AF.Exp)
                    for h2 in range(2):
                        nc.tensor.matmul(avt[h2], vext[:, jc, h * (DH + 1):(h + 1) * (DH + 1)],
                                         p[:, h2 * 512:(h2 + 1) * 512],
                                         start=(jc == 0), stop=(jc == JC - 1))
                for h2 in range(2):
                    avb = avsp.tile([65, 512], F32R, tag="avs")
                    nc.vector.tensor_copy(out=avb, in_=avt[h2])
                    bc = av_ps.tile([64, 512], F32, tag="av")
                    nc.tensor.matmul(bc, ones1[64:65, :], avb[64:65, :], start=True, stop=True)
                    rcp = rp.tile([64, 512], F32, tag="rcp")
                    nc.vector.reciprocal_approx_fast(out=rcp, in_=bc)
                    nc.vector.tensor_mul(
                        out=attn[64 * (h % 2):64 * (h % 2) + 64, h // 2,
                                 h2 * 512:(h2 + 1) * 512],
                        in0=avb[0:64, :].bitcast(F32), in1=rcp)

            def proj(attn, bb):
                for mc in range(CT):
                    for h2 in range(2):
                        ps = qkv_ps.tile([128, 512], F32, tag="q")
                        for t in range(CT):
                            nc.tensor.matmul(ps, wo[:, t, mc * 128:(mc + 1) * 128],
                                             attn[:, t, h2 * 512:(h2 + 1) * 512],
                                             start=(t == 0), stop=(t == CT - 1))
                        ob = obp.tile([128, 512], F32, tag="ob")
                        nc.vector.tensor_copy(out=ob, in_=ps)
                        nc.sync.dma_start(
                            out=out_ext[bb, mc * 128:(mc + 1) * 128, h2 * 512:(h2 + 1) * 512],
                            in_=ob)

            # ---------------- interleaved schedule ----------------
            xn0 = norm(0)
            weight_prep()
            for t in range(CT):
                nc.sync.dma_start(out=xraws[1][:, t, :], in_=x_ext[1, t * 128:(t + 1) * 128, :])
            qT0 = qp.tile([128, CT, N], BF16, tag="qT")
            qkproj(xn0, qT0, range(8))
            vproj(xn0, vexts[0], range(8))
            xn1 = norm(1)

            qT1 = qp.tile([128, CT, N], BF16, tag="qT")
            attn0 = data.tile([128, CT, N], BF16, tag="attn")
            for h in range(HEADS):
                head_attn(h, qT0, vexts[0], attn0)
                # batch 1 projections fill the exp-bound bubbles; k chunks are
                # written into kTp right after batch 0 finishes reading them.
                qkproj(xn1, qT1, [h // 2] if h % 2 == 0 else [4 + (h - 1) // 2])
                vproj(xn1, vexts[1], [h])
            proj(attn0, 0)

            attn1 = data.tile([128, CT, N], BF16, tag="attn")
            for h in range(HEADS):
                head_attn(h, qT1, vexts[1], attn1)
            proj(attn1, 1)
    nc.compile()
    return nc


_NC_CACHE = []


def kernel(x, gamma, mem_kv, w_qkv, w_out, _trace=False):
    x = np.asarray(x, dtype=np.float32)
    gamma = np.asarray(gamma, dtype=np.float32)
    mem_kv = np.asarray(mem_kv, dtype=np.float32)
    w_qkv = np.asarray(w_qkv, dtype=np.float32)
    w_out = np.asarray(w_out, dtype=np.float32)

    b, c, hh, ww = x.shape
    n = hh * ww
    xs = x.reshape(b, c, n)

    wqkvt = np.ascontiguousarray(w_qkv.T)          # [c, 3c]
    wot = np.ascontiguousarray(w_out.T)            # [c, c]
    gammat = np.ascontiguousarray(gamma.reshape(CT, 128).T)  # [128, CT]

    memk = np.zeros((128, HEADS, 128), np.float32)
    memv = np.zeros((128, VW), np.float32)
    for h in range(HEADS):
        r0 = 64 * (h % 2)
        memk[r0:r0 + DH, h, 0:NMEM] = mem_kv[0, h].T      # [dh, nmem]
        memv[0:NMEM, h * (DH + 1):h * (DH + 1) + DH] = mem_kv[1, h]
        memv[0:NMEM, h * (DH + 1) + DH] = 1.0

    if not _NC_CACHE:
        _NC_CACHE.append(_build())
    nc = _NC_CACHE[0]

    in_maps = []
    for core in range(NCORES):
        in_maps.append({
            "x": np.ascontiguousarray(xs[core * PB:(core + 1) * PB]),
            "wqkvt": wqkvt,
            "wot": wot,
            "gammat": gammat,
            "memk": memk,
            "memv": memv,
        })
    res = run_bass_kernel_spmd(nc, in_maps, core_ids=list(range(NCORES)), trace=_trace)
    out = np.concatenate([res.results[core]["out"] for core in range(NCORES)], axis=0)
    kernel.last_result = res
    return out.reshape(b, c, hh, ww)


# revision 26
# speedup vs baseline: 1.0887x; 1.0063x over previous
"""Trainium2 Bass kernel for nn_Attention_7945689497706.

Distribution: data-parallel over batch, 2 batch elements per core, weights
replicated, no collectives.

Per-core layout:
  - RMSNorm via ones-matmul partition reduction, gamma folded into weights.
  - q^T,k^T in [o, n] fp32r; v in [n, o] feeding a bf16 [v|1] (j, 65) tile.
  - Attention transposed (j on psum partitions): sim_T = kTpad^T qT with K
    zero-padded to 128; exp on ACT at [128,1024] grain; av lhsT = vext so the
    ones column accumulates softmax denominators; normalization = K=1 matmul
    broadcast + DVE fast-reciprocal + multiply.
  - mem_kv + padding in a 9th j-chunk (zero k-cols / zero v-rows make the
    padded lanes contribute nothing).
  - The two batch elements are software-pipelined: batch 1's norm/projections
    are emitted inside batch 0's attention loop (per-head kTp handoff) so the
    PE fills the ACT-bound exp bubbles.
"""

import numpy as np

import concourse.bass as bass
import concourse.mybir as mybir
import concourse.tile as tile
from concourse import bacc
from concourse.bass_utils import run_bass_kernel_spmd

F32 = mybir.dt.float32
F32R = mybir.dt.float32r
BF16 = mybir.dt.bfloat16
AF = mybir.ActivationFunctionType

NCORES = 8
B = 16
C = 512
N = 1024          # pixels = 32*32
HEADS = 8
DH = 64
NMEM = 4
PB = B // NCORES  # batch elements per core
CT = C // 128     # channel partition-tiles
JC = 9            # j chunks: 8 pixel chunks + 1 (mem + zero pad)
VW = HEADS * (DH + 1)  # vext width: per head [v | ones] = 65


def _build():
    nc = bacc.Bacc()
    x_ext = nc.declare_dram_parameter("x", [PB, C, N], F32, isOutput=False)
    wqkvt_ext = nc.declare_dram_parameter("wqkvt", [C, 3 * C], F32, isOutput=False)
    wot_ext = nc.declare_dram_parameter("wot", [C, C], F32, isOutput=False)
    gammat_ext = nc.declare_dram_parameter("gammat", [128, CT], F32, isOutput=False)
    memk_ext = nc.declare_dram_parameter("memk", [128, HEADS, NMEM], F32, isOutput=False)
    memv_ext = nc.declare_dram_parameter("memv", [128, 2, VW], F32, isOutput=False)
    out_ext = nc.declare_dram_parameter("out", [PB, C, N], F32, isOutput=True)

    with tile.TileContext(nc) as tc:
        with (
            tc.tile_pool(name="const", bufs=1) as const,
            tc.tile_pool(name="wstage", bufs=1) as wstage,
            tc.tile_pool(name="xp", bufs=2) as xp,
            tc.tile_pool(name="data", bufs=1) as data,
            tc.tile_pool(name="qp", bufs=2) as qp,
            tc.tile_pool(name="pp", bufs=4) as pp,
            tc.tile_pool(name="pm", bufs=4) as pm,
            tc.tile_pool(name="avs", bufs=2) as avsp,
            tc.tile_pool(name="rp", bufs=2) as rp,
            tc.tile_pool(name="ob", bufs=2) as obp,
            tc.tile_pool(name="qkv_ps", bufs=2, space="PSUM") as qkv_ps,
            tc.tile_pool(name="sim_ps", bufs=2, space="PSUM") as sim_ps,
            tc.tile_pool(name="av_ps", bufs=2, space="PSUM") as av_ps,
        ):
            # ------------ batch-0 x load first (weights stream behind it) -------
            xraws = []
            for bb in range(PB):
                xr = xp.tile([128, CT, N], F32, tag="xraw")
                xraws.append(xr)
            for t in range(CT):
                nc.sync.dma_start(out=xraws[0][:, t, :], in_=x_ext[0, t * 128:(t + 1) * 128, :])

            # ---------------- per-core constants ----------------
            wqkv = const.tile([128, CT, 3 * C], BF16, tag="wqkv")
            wo = const.tile([128, CT, C], BF16, tag="wo")
            g1 = const.tile([128, CT], F32, tag="g1")
            g1q = const.tile([128, CT], F32, tag="g1q")
            ones128 = const.tile([128, 128], BF16, tag="ones128")
            ones1 = const.tile([128, 64], F32R, tag="ones1")
            kTp = const.tile([128, HEADS, 1028], BF16, tag="kTp")
            vextA = const.tile([128, 8, VW], BF16, tag="vextA")
            vextB = const.tile([128, 8, VW], BF16, tag="vextB")
            vmem = const.tile([128, 2, VW], BF16, tag="vmem")
            vexts = [vextA, vextB]

            gsb = const.tile([128, CT], F32, tag="gsb")
            nc.sync.dma_start(out=gsb, in_=gammat_ext[:, :])
            nc.scalar.activation(out=g1, in_=gsb, func=AF.Copy, bias=1.0)
            nc.scalar.activation(out=g1q, in_=gsb, func=AF.Copy, bias=1.0, scale=1.0)
            nc.scalar.mul(out=g1q, in_=g1q, mul=DH ** -0.5)

            nc.vector.memset(ones128, 1.0)
            nc.vector.memset(ones1.bitcast(F32), 1.0)

            def weight_prep():
                nc.gpsimd.memset(kTp, 0.0)
                for t in range(CT):
                    ws = wstage.tile([128, 3 * C], F32, tag="ws")
                    nc.sync.dma_start(out=ws, in_=wqkvt_ext[t * 128:(t + 1) * 128, :])
                    nc.vector.tensor_scalar_mul(
                        out=wqkv[:, t, 0:C], in0=ws[:, 0:C], scalar1=g1q[:, t:t + 1])
                    nc.vector.tensor_scalar_mul(
                        out=wqkv[:, t, C:3 * C], in0=ws[:, C:3 * C], scalar1=g1[:, t:t + 1])
                for t in range(CT):
                    ws = wstage.tile([128, 3 * C], F32, tag="ws")
                    nc.sync.dma_start(out=ws[:, 0:C], in_=wot_ext[t * 128:(t + 1) * 128, :])
                    nc.vector.tensor_copy(out=wo[:, t, :], in_=ws[:, 0:C])
                # mem_kv constants
                ws = wstage.tile([128, 3 * C], F32, tag="ws")
                nc.sync.dma_start(out=ws[:, 0:HEADS * NMEM],
                                  in_=memk_ext[:, :, :].rearrange("p h c -> p (h c)"))
                nc.sync.dma_start(out=ws[:, HEADS * NMEM:HEADS * NMEM + 2 * VW],
                                  in_=memv_ext[:, :, :].rearrange("p g c -> p (g c)"))
                nc.vector.tensor_copy(
                    out=kTp[:, :, 1024:1028],
                    in_=ws[:, 0:HEADS * NMEM].rearrange("p (h c) -> p h c", c=NMEM))
                nc.vector.tensor_copy(
                    out=vmem,
                    in_=ws[:, HEADS * NMEM:HEADS * NMEM + 2 * VW].rearrange("p (g c) -> p g c", c=VW))
                for v in vexts:
                    oc = v[:, :, :].rearrange("p j (h c) -> p j h c", c=DH + 1)[:, :, :, DH:DH + 1]
                    nc.gpsimd.memset(oc, 1.0)

            # ---------------- pipeline stages ----------------
            def norm(bb):
                """x -> xn (fp32r, per-pixel normalized)."""
                xraw = xraws[bb]
                xsq = data.tile([128, CT, N], BF16, tag="xsq")
                for t in range(CT):
                    nc.vector.tensor_mul(out=xsq[:, t, :], in0=xraw[:, t, :], in1=xraw[:, t, :])
                ss = sim_ps.tile([128, N], F32, tag="sim")
                for h2 in range(2):
                    for t in range(CT):
                        nc.tensor.matmul(ss[:, h2 * 512:(h2 + 1) * 512], ones128,
                                         xsq[:, t, h2 * 512:(h2 + 1) * 512],
                                         start=(t == 0), stop=(t == CT - 1))
                sroot = data.tile([128, N], F32, tag="sroot")
                nc.scalar.activation(out=sroot, in_=ss, func=AF.Sqrt, scale=1.0 / C)
                snorm = data.tile([128, N], F32, tag="snorm")
                nc.vector.reciprocal_approx_fast(out=snorm, in_=sroot)
                xn = data.tile([128, CT, N], BF16, tag="xn" + str(bb))
                for t in range(CT):
                    nc.vector.tensor_mul(out=xn[:, t, :], in0=xraw[:, t, :], in1=snorm)
                return xn

            def qkproj(xn, qT, mcs):
                """o-chunks mcs of the q/k projection; k goes into kTp (padded)."""
                for mc in mcs:
                    for h2 in range(2):
                        ps = qkv_ps.tile([128, 512], F32, tag="q")
                        for t in range(CT):
                            nc.tensor.matmul(ps, wqkv[:, t, mc * 128:(mc + 1) * 128],
                                             xn[:, t, h2 * 512:(h2 + 1) * 512],
                                             start=(t == 0), stop=(t == CT - 1))
                        if mc < 4:
                            nc.vector.tensor_copy(out=qT[:, mc, h2 * 512:(h2 + 1) * 512], in_=ps)
                        else:
                            h0, h1 = 2 * (mc - 4), 2 * (mc - 4) + 1
                            nc.vector.tensor_copy(
                                out=kTp[0:64, h0, h2 * 512:(h2 + 1) * 512], in_=ps[0:64, :])
                            nc.vector.tensor_copy(
                                out=kTp[64:128, h1, h2 * 512:(h2 + 1) * 512], in_=ps[64:128, :])

            def vproj(xn, vext, ics):
                for ic in ics:
                    ps = qkv_ps.tile([128, 512], F32, tag="q")
                    for t in range(CT):
                        nc.tensor.matmul(ps, xn[:, t, ic * 128:(ic + 1) * 128],
                                         wqkv[:, t, 2 * C:3 * C],
                                         start=(t == 0), stop=(t == CT - 1))
                    ps_h = ps[:, :].rearrange("p (h c) -> p h c", c=DH)
                    vdst = vext[:, ic, :].rearrange("p (h c) -> p h c", c=DH + 1)[:, :, 0:DH]
                    nc.vector.tensor_copy(out=vdst, in_=ps_h)

            def head_attn(h, qT, vext, attn, pmem):
                av0 = av_ps.tile([65, 512], F32, tag="av")
                av1 = av_ps.tile([65, 512], F32, tag="av")
                avt = (av0, av1)
                for jc in range(8):
                    st = sim_ps.tile([128, N], F32, tag="sim")
                    for h2 in range(2):
                        nc.tensor.matmul(st[:, h2 * 512:(h2 + 1) * 512],
                                         kTp[:, h, jc * 128:(jc + 1) * 128],
                                         qT[:, h // 2, h2 * 512:(h2 + 1) * 512],
                                         start=True, stop=True)
                    p = pp.tile([128, N], BF16, tag="p")
                    nc.scalar.activation(out=p, in_=st, func=AF.Exp)
                    for h2 in range(2):
                        nc.tensor.matmul(avt[h2], vext[:, jc, h * (DH + 1):(h + 1) * (DH + 1)],
                                         p[:, h2 * 512:(h2 + 1) * 512],
                                         start=(jc == 0), stop=False)
                # mem_kv contribution from the shared per-4-head exp tiles
                g, r0 = h // 4, 32 * (h % 4)
                for h2 in range(2):
                    nc.tensor.matmul(avt[h2],
                                     vmem[r0:r0 + NMEM, g, (h % 4) * (DH + 1):(h % 4 + 1) * (DH + 1)],
                                     pmem[g][r0:r0 + NMEM, h2 * 512:(h2 + 1) * 512],
                                     start=False, stop=True, tile_position=(r0, 0))
                for h2 in range(2):
                    avb = avsp.tile([65, 512], F32R, tag="avs")
                    nc.vector.tensor_copy(out=avb, in_=avt[h2])
                    bc = av_ps.tile([64, 512], F32, tag="av")
                    nc.tensor.matmul(bc, ones1[64:65, :], avb[64:65, :], start=True, stop=True)
                    rcp = rp.tile([64, 512], F32, tag="rcp")
                    nc.vector.reciprocal_approx_fast(out=rcp, in_=bc)
                    nc.vector.tensor_mul(
                        out=attn[64 * (h % 2):64 * (h % 2) + 64, h // 2,
                                 h2 * 512:(h2 + 1) * 512],
                        in0=avb[0:64, :].bitcast(F32), in1=rcp)

            def proj(attn, bb):
                for mc in range(CT):
                    for h2 in range(2):
                        ps = qkv_ps.tile([128, 512], F32, tag="q")
                        for t in range(CT):
                            nc.tensor.matmul(ps, wo[:, t, mc * 128:(mc + 1) * 128],
                                             attn[:, t, h2 * 512:(h2 + 1) * 512],
                                             start=(t == 0), stop=(t == CT - 1))
                        ob = obp.tile([128, 512], F32, tag="ob")
                        nc.vector.tensor_copy(out=ob, in_=ps)
                        nc.sync.dma_start(
                            out=out_ext[bb, mc * 128:(mc + 1) * 128, h2 * 512:(h2 + 1) * 512],
                            in_=ob)

            def mem_sims(qT):
                pms = []
                for g in range(2):
                    st = sim_ps.tile([128, N], F32, tag="sim")
                    for h4 in range(4):
                        h = 4 * g + h4
                        for h2 in range(2):
                            nc.tensor.matmul(st[32 * h4:32 * h4 + NMEM, h2 * 512:(h2 + 1) * 512],
                                             kTp[:, h, 1024:1028],
                                             qT[:, h // 2, h2 * 512:(h2 + 1) * 512],
                                             start=True, stop=True, tile_position=(0, 32 * h4))
                    pmt = pm.tile([128, N], BF16, tag="pm")
                    nc.scalar.activation(out=pmt, in_=st, func=AF.Exp)
                    pms.append(pmt)
                return pms

            # ---------------- interleaved schedule ----------------
            xn0 = norm(0)
            weight_prep()
            for t in range(CT):
                nc.sync.dma_start(out=xraws[1][:, t, :], in_=x_ext[1, t * 128:(t + 1) * 128, :])
            qT0 = qp.tile([128, CT, N], BF16, tag="qT")
            qkproj(xn0, qT0, range(8))
            vproj(xn0, vexts[0], range(8))
            xn1 = norm(1)

            pmem0 = mem_sims(qT0)
            qT1 = qp.tile([128, CT, N], BF16, tag="qT")
            attn0 = data.tile([128, CT, N], BF16, tag="attn")
            for h in range(HEADS):
                head_attn(h, qT0, vexts[0], attn0, pmem0)
                # batch 1 projections fill the exp-bound bubbles; k chunks are
                # written into kTp right after batch 0 finishes reading them.
                qkproj(xn1, qT1, [h // 2] if h % 2 == 0 else [4 + (h - 1) // 2])
                vproj(xn1, vexts[1], [h])
            proj(attn0, 0)

            pmem1 = mem_sims(qT1)
            attn1 = data.tile([128, CT, N], BF16, tag="attn")
            for h in range(HEADS):
                head_attn(h, qT1, vexts[1], attn1, pmem1)
            proj(attn1, 1)
    nc.compile()
    return nc


_NC_CACHE = []


def kernel(x, gamma, mem_kv, w_qkv, w_out, _trace=False):
    x = np.asarray(x, dtype=np.float32)
    gamma = np.asarray(gamma, dtype=np.float32)
    mem_kv = np.asarray(mem_kv, dtype=np.float32)
    w_qkv = np.asarray(w_qkv, dtype=np.float32)
    w_out = np.asarray(w_out, dtype=np.float32)

    b, c, hh, ww = x.shape
    n = hh * ww
    xs = x.reshape(b, c, n)

    wqkvt = np.ascontiguousarray(w_qkv.T)          # [c, 3c]
    wot = np.ascontiguousarray(w_out.T)            # [c, c]
    gammat = np.ascontiguousarray(gamma.reshape(CT, 128).T)  # [128, CT]

    memk = np.zeros((128, HEADS, NMEM), np.float32)
    memv = np.zeros((128, 2, VW), np.float32)
    for h in range(HEADS):
        r0 = 64 * (h % 2)
        memk[r0:r0 + DH, h, 0:NMEM] = mem_kv[0, h].T      # [dh, nmem]
        g, r1, c0 = h // 4, 32 * (h % 4), (h % 4) * (DH + 1)
        memv[r1:r1 + NMEM, g, c0:c0 + DH] = mem_kv[1, h]
        memv[r1:r1 + NMEM, g, c0 + DH] = 1.0

    if not _NC_CACHE:
        _NC_CACHE.append(_build())
    nc = _NC_CACHE[0]

    in_maps = []
    for core in range(NCORES):
        in_maps.append({
            "x": np.ascontiguousarray(xs[core * PB:(core + 1) * PB]),
            "wqkvt": wqkvt,
            "wot": wot,
            "gammat": gammat,
            "memk": memk,
            "memv": memv,
        })
    res = run_bass_kernel_spmd(nc, in_maps, core_ids=list(range(NCORES)), trace=_trace)
    out = np.concatenate([res.results[core]["out"] for core in range(NCORES)], axis=0)
    kernel.last_result = res
    return out.reshape(b, c, hh, ww)


# revision 27
# speedup vs baseline: 1.1123x; 1.0217x over previous
"""Trainium2 Bass kernel for nn_Attention_7945689497706.

Distribution: data-parallel over batch, 2 batch elements per core, weights
replicated, no collectives.

Per-core layout:
  - RMSNorm via ones-matmul partition reduction, gamma folded into weights.
  - q^T,k^T in [o, n] fp32r; v in [n, o] feeding a bf16 [v|1] (j, 65) tile.
  - Attention transposed (j on psum partitions): sim_T = kTpad^T qT with K
    zero-padded to 128; exp on ACT at [128,1024] grain; av lhsT = vext so the
    ones column accumulates softmax denominators; normalization = K=1 matmul
    broadcast + DVE fast-reciprocal + multiply.
  - mem_kv + padding in a 9th j-chunk (zero k-cols / zero v-rows make the
    padded lanes contribute nothing).
  - The two batch elements are software-pipelined: batch 1's norm/projections
    are emitted inside batch 0's attention loop (per-head kTp handoff) so the
    PE fills the ACT-bound exp bubbles.
"""

import numpy as np

import concourse.bass as bass
import concourse.mybir as mybir
import concourse.tile as tile
from concourse import bacc
from concourse.bass_utils import run_bass_kernel_spmd

F32 = mybir.dt.float32
F32R = mybir.dt.float32r
BF16 = mybir.dt.bfloat16
AF = mybir.ActivationFunctionType

NCORES = 8
B = 16
C = 512
N = 1024          # pixels = 32*32
HEADS = 8
DH = 64
NMEM = 4
PB = B // NCORES  # batch elements per core
CT = C // 128     # channel partition-tiles
JC = 9            # j chunks: 8 pixel chunks + 1 (mem + zero pad)
VW = HEADS * (DH + 1)  # vext width: per head [v | ones] = 65


def _build():
    nc = bacc.Bacc()
    x_ext = nc.declare_dram_parameter("x", [PB, C, N], F32, isOutput=False)
    wqkvt_ext = nc.declare_dram_parameter("wqkvt", [C, 3 * C], F32, isOutput=False)
    wot_ext = nc.declare_dram_parameter("wot", [C, C], F32, isOutput=False)
    gammat_ext = nc.declare_dram_parameter("gammat", [128, CT], F32, isOutput=False)
    memk_ext = nc.declare_dram_parameter("memk", [128, HEADS, NMEM], F32, isOutput=False)
    memv_ext = nc.declare_dram_parameter("memv", [128, 2, VW], F32, isOutput=False)
    out_ext = nc.declare_dram_parameter("out", [PB, C, N], F32, isOutput=True)

    with tile.TileContext(nc) as tc:
        with (
            tc.tile_pool(name="const", bufs=1) as const,
            tc.tile_pool(name="wstage", bufs=2) as wstage,
            tc.tile_pool(name="xp", bufs=2) as xp,
            tc.tile_pool(name="data", bufs=1) as data,
            tc.tile_pool(name="qp", bufs=2) as qp,
            tc.tile_pool(name="pp", bufs=4) as pp,
            tc.tile_pool(name="pm", bufs=4) as pm,
            tc.tile_pool(name="avs", bufs=2) as avsp,
            tc.tile_pool(name="rp", bufs=2) as rp,
            tc.tile_pool(name="ob", bufs=2) as obp,
            tc.tile_pool(name="qkv_ps", bufs=2, space="PSUM") as qkv_ps,
            tc.tile_pool(name="sim_ps", bufs=2, space="PSUM") as sim_ps,
            tc.tile_pool(name="av_ps", bufs=2, space="PSUM") as av_ps,
        ):
            # ------------ batch-0 x load first (weights stream behind it) -------
            xraws = []
            for bb in range(PB):
                xr = xp.tile([128, CT, N], F32, tag="xraw")
                xraws.append(xr)
            for t in range(CT):
                nc.sync.dma_start(out=xraws[0][:, t, :], in_=x_ext[0, t * 128:(t + 1) * 128, :])

            # ---------------- per-core constants ----------------
            wqkv = const.tile([128, CT, 3 * C], BF16, tag="wqkv")
            wo = const.tile([128, CT, C], BF16, tag="wo")
            g1 = const.tile([128, CT], F32, tag="g1")
            g1q = const.tile([128, CT], F32, tag="g1q")
            ones128 = const.tile([128, 128], BF16, tag="ones128")
            ones1 = const.tile([128, 64], F32R, tag="ones1")
            kTp = const.tile([128, HEADS, 1028], BF16, tag="kTp")
            vextA = const.tile([128, 8, VW], BF16, tag="vextA")
            vextB = const.tile([128, 8, VW], BF16, tag="vextB")
            vmem = const.tile([128, 2, VW], BF16, tag="vmem")
            vexts = [vextA, vextB]

            gsb = const.tile([128, CT], F32, tag="gsb")
            nc.sync.dma_start(out=gsb, in_=gammat_ext[:, :])
            nc.scalar.activation(out=g1, in_=gsb, func=AF.Copy, bias=1.0)
            nc.scalar.activation(out=g1q, in_=gsb, func=AF.Copy, bias=1.0, scale=1.0)
            nc.scalar.mul(out=g1q, in_=g1q, mul=DH ** -0.5)

            nc.vector.memset(ones128, 1.0)
            nc.vector.memset(ones1.bitcast(F32), 1.0)

            def weight_prep():
                nc.gpsimd.memset(kTp, 0.0)
                for t in range(CT):
                    ws = wstage.tile([128, 3 * C], F32, tag="ws")
                    nc.sync.dma_start(out=ws, in_=wqkvt_ext[t * 128:(t + 1) * 128, :])
                    nc.vector.tensor_scalar_mul(
                        out=wqkv[:, t, 0:C], in0=ws[:, 0:C], scalar1=g1q[:, t:t + 1])
                    nc.vector.tensor_scalar_mul(
                        out=wqkv[:, t, C:3 * C], in0=ws[:, C:3 * C], scalar1=g1[:, t:t + 1])
                for t in range(CT):
                    ws = wstage.tile([128, 3 * C], F32, tag="ws")
                    nc.sync.dma_start(out=ws[:, 0:C], in_=wot_ext[t * 128:(t + 1) * 128, :])
                    nc.vector.tensor_copy(out=wo[:, t, :], in_=ws[:, 0:C])
                # mem_kv constants
                ws = wstage.tile([128, 3 * C], F32, tag="ws")
                nc.sync.dma_start(out=ws[:, 0:HEADS * NMEM],
                                  in_=memk_ext[:, :, :].rearrange("p h c -> p (h c)"))
                nc.sync.dma_start(out=ws[:, HEADS * NMEM:HEADS * NMEM + 2 * VW],
                                  in_=memv_ext[:, :, :].rearrange("p g c -> p (g c)"))
                nc.vector.tensor_copy(
                    out=kTp[:, :, 1024:1028],
                    in_=ws[:, 0:HEADS * NMEM].rearrange("p (h c) -> p h c", c=NMEM))
                nc.vector.tensor_copy(
                    out=vmem,
                    in_=ws[:, HEADS * NMEM:HEADS * NMEM + 2 * VW].rearrange("p (g c) -> p g c", c=VW))
                for v in vexts:
                    oc = v[:, :, :].rearrange("p j (h c) -> p j h c", c=DH + 1)[:, :, :, DH:DH + 1]
                    nc.gpsimd.memset(oc, 1.0)

            # ---------------- pipeline stages ----------------
            def norm(bb):
                """x -> xn (fp32r, per-pixel normalized)."""
                xraw = xraws[bb]
                xsq = data.tile([128, CT, N], BF16, tag="xsq")
                for t in range(CT):
                    nc.vector.tensor_mul(out=xsq[:, t, :], in0=xraw[:, t, :], in1=xraw[:, t, :])
                ss = sim_ps.tile([128, N], F32, tag="sim")
                for h2 in range(2):
                    for t in range(CT):
                        nc.tensor.matmul(ss[:, h2 * 512:(h2 + 1) * 512], ones128,
                                         xsq[:, t, h2 * 512:(h2 + 1) * 512],
                                         start=(t == 0), stop=(t == CT - 1))
                sroot = data.tile([128, N], F32, tag="sroot")
                nc.scalar.activation(out=sroot, in_=ss, func=AF.Sqrt, scale=1.0 / C)
                snorm = data.tile([128, N], F32, tag="snorm")
                nc.vector.reciprocal_approx_fast(out=snorm, in_=sroot)
                xn = data.tile([128, CT, N], BF16, tag="xn" + str(bb))
                for t in range(CT):
                    nc.vector.tensor_mul(out=xn[:, t, :], in0=xraw[:, t, :], in1=snorm)
                return xn

            def qkproj(xn, qT, mcs):
                """o-chunks mcs of the q/k projection; k goes into kTp (padded)."""
                for mc in mcs:
                    for h2 in range(2):
                        ps = qkv_ps.tile([128, 512], F32, tag="q")
                        for t in range(CT):
                            nc.tensor.matmul(ps, wqkv[:, t, mc * 128:(mc + 1) * 128],
                                             xn[:, t, h2 * 512:(h2 + 1) * 512],
                                             start=(t == 0), stop=(t == CT - 1))
                        if mc < 4:
                            nc.vector.tensor_copy(out=qT[:, mc, h2 * 512:(h2 + 1) * 512], in_=ps)
                        else:
                            h0, h1 = 2 * (mc - 4), 2 * (mc - 4) + 1
                            nc.vector.tensor_copy(
                                out=kTp[0:64, h0, h2 * 512:(h2 + 1) * 512], in_=ps[0:64, :])
                            nc.vector.tensor_copy(
                                out=kTp[64:128, h1, h2 * 512:(h2 + 1) * 512], in_=ps[64:128, :])

            def vproj(xn, vext, ics):
                for ic in ics:
                    ps = qkv_ps.tile([128, 512], F32, tag="q")
                    for t in range(CT):
                        nc.tensor.matmul(ps, xn[:, t, ic * 128:(ic + 1) * 128],
                                         wqkv[:, t, 2 * C:3 * C],
                                         start=(t == 0), stop=(t == CT - 1))
                    ps_h = ps[:, :].rearrange("p (h c) -> p h c", c=DH)
                    vdst = vext[:, ic, :].rearrange("p (h c) -> p h c", c=DH + 1)[:, :, 0:DH]
                    nc.vector.tensor_copy(out=vdst, in_=ps_h)

            def head_attn(h, qT, vext, attn, pmem):
                av0 = av_ps.tile([65, 512], F32, tag="av")
                av1 = av_ps.tile([65, 512], F32, tag="av")
                avt = (av0, av1)
                for jc in range(8):
                    st = sim_ps.tile([128, N], F32, tag="sim")
                    for h2 in range(2):
                        nc.tensor.matmul(st[:, h2 * 512:(h2 + 1) * 512],
                                         kTp[:, h, jc * 128:(jc + 1) * 128],
                                         qT[:, h // 2, h2 * 512:(h2 + 1) * 512],
                                         start=True, stop=True)
                    p = pp.tile([128, N], BF16, tag="p")
                    nc.scalar.activation(out=p, in_=st, func=AF.Exp)
                    for h2 in range(2):
                        nc.tensor.matmul(avt[h2], vext[:, jc, h * (DH + 1):(h + 1) * (DH + 1)],
                                         p[:, h2 * 512:(h2 + 1) * 512],
                                         start=(jc == 0), stop=False)
                # mem_kv contribution from the shared per-4-head exp tiles
                g, r0 = h // 4, 32 * (h % 4)
                for h2 in range(2):
                    nc.tensor.matmul(avt[h2],
                                     vmem[r0:r0 + NMEM, g, (h % 4) * (DH + 1):(h % 4 + 1) * (DH + 1)],
                                     pmem[g][r0:r0 + NMEM, h2 * 512:(h2 + 1) * 512],
                                     start=False, stop=True, tile_position=(r0, 0))
                for h2 in range(2):
                    avb = avsp.tile([65, 512], F32R, tag="avs")
                    nc.vector.tensor_copy(out=avb, in_=avt[h2])
                    bc = av_ps.tile([64, 512], F32, tag="av")
                    nc.tensor.matmul(bc, ones1[64:65, :], avb[64:65, :], start=True, stop=True)
                    rcp = rp.tile([64, 512], F32, tag="rcp")
                    nc.vector.reciprocal_approx_fast(out=rcp, in_=bc)
                    nc.vector.tensor_mul(
                        out=attn[64 * (h % 2):64 * (h % 2) + 64, h // 2,
                                 h2 * 512:(h2 + 1) * 512],
                        in0=avb[0:64, :].bitcast(F32), in1=rcp)

            def proj(attn, bb):
                for mc in range(CT):
                    for h2 in range(2):
                        ps = qkv_ps.tile([128, 512], F32, tag="q")
                        for t in range(CT):
                            nc.tensor.matmul(ps, wo[:, t, mc * 128:(mc + 1) * 128],
                                             attn[:, t, h2 * 512:(h2 + 1) * 512],
                                             start=(t == 0), stop=(t == CT - 1))
                        ob = obp.tile([128, 512], F32, tag="ob")
                        nc.vector.tensor_copy(out=ob, in_=ps)
                        nc.sync.dma_start(
                            out=out_ext[bb, mc * 128:(mc + 1) * 128, h2 * 512:(h2 + 1) * 512],
                            in_=ob)

            def mem_sims(qT):
                pms = []
                for g in range(2):
                    st = sim_ps.tile([128, N], F32, tag="sim")
                    for h4 in range(4):
                        h = 4 * g + h4
                        for h2 in range(2):
                            nc.tensor.matmul(st[32 * h4:32 * h4 + NMEM, h2 * 512:(h2 + 1) * 512],
                                             kTp[:, h, 1024:1028],
                                             qT[:, h // 2, h2 * 512:(h2 + 1) * 512],
                                             start=True, stop=True, tile_position=(0, 32 * h4))
                    pmt = pm.tile([128, N], BF16, tag="pm")
                    nc.scalar.activation(out=pmt, in_=st, func=AF.Exp)
                    pms.append(pmt)
                return pms

            # ---------------- interleaved schedule ----------------
            xn0 = norm(0)
            weight_prep()
            for t in range(CT):
                nc.sync.dma_start(out=xraws[1][:, t, :], in_=x_ext[1, t * 128:(t + 1) * 128, :])
            qT0 = qp.tile([128, CT, N], BF16, tag="qT")
            qkproj(xn0, qT0, range(8))
            vproj(xn0, vexts[0], range(8))
            xn1 = norm(1)

            pmem0 = mem_sims(qT0)
            qT1 = qp.tile([128, CT, N], BF16, tag="qT")
            attn0 = data.tile([128, CT, N], BF16, tag="attn")
            for h in range(HEADS):
                head_attn(h, qT0, vexts[0], attn0, pmem0)
                # batch 1 projections fill the exp-bound bubbles; k chunks are
                # written into kTp right after batch 0 finishes reading them.
                qkproj(xn1, qT1, [h // 2] if h % 2 == 0 else [4 + (h - 1) // 2])
                vproj(xn1, vexts[1], [h])
            proj(attn0, 0)

            pmem1 = mem_sims(qT1)
            attn1 = data.tile([128, CT, N], BF16, tag="attn")
            for h in range(HEADS):
                head_attn(h, qT1, vexts[1], attn1, pmem1)
            proj(attn1, 1)
    nc.compile()
    return nc


_NC_CACHE = []


def kernel(x, gamma, mem_kv, w_qkv, w_out, _trace=False):
    x = np.asarray(x, dtype=np.float32)
    gamma = np.asarray(gamma, dtype=np.float32)
    mem_kv = np.asarray(mem_kv, dtype=np.float32)
    w_qkv = np.asarray(w_qkv, dtype=np.float32)
    w_out = np.asarray(w_out, dtype=np.float32)

    b, c, hh, ww = x.shape
    n = hh * ww
    xs = x.reshape(b, c, n)

    wqkvt = np.ascontiguousarray(w_qkv.T)          # [c, 3c]
    wot = np.ascontiguousarray(w_out.T)            # [c, c]
    gammat = np.ascontiguousarray(gamma.reshape(CT, 128).T)  # [128, CT]

    memk = np.zeros((128, HEADS, NMEM), np.float32)
    memv = np.zeros((128, 2, VW), np.float32)
    for h in range(HEADS):
        r0 = 64 * (h % 2)
        memk[r0:r0 + DH, h, 0:NMEM] = mem_kv[0, h].T      # [dh, nmem]
        g, r1, c0 = h // 4, 32 * (h % 4), (h % 4) * (DH + 1)
        memv[r1:r1 + NMEM, g, c0:c0 + DH] = mem_kv[1, h]
        memv[r1:r1 + NMEM, g, c0 + DH] = 1.0

    if not _NC_CACHE:
        _NC_CACHE.append(_build())
    nc = _NC_CACHE[0]

    in_maps = []
    for core in range(NCORES):
        in_maps.append({
            "x": np.ascontiguousarray(xs[core * PB:(core + 1) * PB]),
            "wqkvt": wqkvt,
            "wot": wot,
            "gammat": gammat,
            "memk": memk,
            "memv": memv,
        })
    res = run_bass_kernel_spmd(nc, in_maps, core_ids=list(range(NCORES)), trace=_trace)
    out = np.concatenate([res.results[core]["out"] for core in range(NCORES)], axis=0)
    kernel.last_result = res
    return out.reshape(b, c, hh, ww)


# revision 28
# speedup vs baseline: 1.1186x; 1.0057x over previous
"""Trainium2 Bass kernel for nn_Attention_7945689497706.

Distribution: data-parallel over batch, 2 batch elements per core, weights
replicated, no collectives.

Per-core layout:
  - RMSNorm via ones-matmul partition reduction, gamma folded into weights.
  - q^T,k^T in [o, n] fp32r; v in [n, o] feeding a bf16 [v|1] (j, 65) tile.
  - Attention transposed (j on psum partitions): sim_T = kTpad^T qT with K
    zero-padded to 128; exp on ACT at [128,1024] grain; av lhsT = vext so the
    ones column accumulates softmax denominators; normalization = K=1 matmul
    broadcast + DVE fast-reciprocal + multiply.
  - mem_kv + padding in a 9th j-chunk (zero k-cols / zero v-rows make the
    padded lanes contribute nothing).
  - The two batch elements are software-pipelined: batch 1's norm/projections
    are emitted inside batch 0's attention loop (per-head kTp handoff) so the
    PE fills the ACT-bound exp bubbles.
"""

import numpy as np

import concourse.bass as bass
import concourse.mybir as mybir
import concourse.tile as tile
from concourse import bacc
from concourse.bass_utils import run_bass_kernel_spmd

F32 = mybir.dt.float32
F32R = mybir.dt.float32r
BF16 = mybir.dt.bfloat16
AF = mybir.ActivationFunctionType

NCORES = 8
B = 16
C = 512
N = 1024          # pixels = 32*32
HEADS = 8
DH = 64
NMEM = 4
PB = B // NCORES  # batch elements per core
CT = C // 128     # channel partition-tiles
JC = 9            # j chunks: 8 pixel chunks + 1 (mem + zero pad)
VW = HEADS * (DH + 1)  # vext width: per head [v | ones] = 65


def _build():
    nc = bacc.Bacc()
    x_ext = nc.declare_dram_parameter("x", [PB, C, N], F32, isOutput=False)
    wqkvt_ext = nc.declare_dram_parameter("wqkvt", [C, 3 * C], F32, isOutput=False)
    wot_ext = nc.declare_dram_parameter("wot", [C, C], F32, isOutput=False)
    gammat_ext = nc.declare_dram_parameter("gammat", [128, CT], F32, isOutput=False)
    memk_ext = nc.declare_dram_parameter("memk", [128, HEADS, NMEM], F32, isOutput=False)
    memv_ext = nc.declare_dram_parameter("memv", [128, 2, VW], F32, isOutput=False)
    out_ext = nc.declare_dram_parameter("out", [PB, C, N], F32, isOutput=True)

    with tile.TileContext(nc) as tc:
        with (
            tc.tile_pool(name="const", bufs=1) as const,
            tc.tile_pool(name="wstage", bufs=2) as wstage,
            tc.tile_pool(name="xp", bufs=2) as xp,
            tc.tile_pool(name="data", bufs=1) as data,
            tc.tile_pool(name="qp", bufs=2) as qp,
            tc.tile_pool(name="pp", bufs=4) as pp,
            tc.tile_pool(name="pm", bufs=4) as pm,
            tc.tile_pool(name="avs", bufs=3) as avsp,
            tc.tile_pool(name="rp", bufs=3) as rp,
            tc.tile_pool(name="ob", bufs=2) as obp,
            tc.tile_pool(name="qkv_ps", bufs=2, space="PSUM") as qkv_ps,
            tc.tile_pool(name="sim_ps", bufs=2, space="PSUM") as sim_ps,
            tc.tile_pool(name="av_ps", bufs=2, space="PSUM") as av_ps,
        ):
            # ------------ batch-0 x load first (weights stream behind it) -------
            xraws = []
            for bb in range(PB):
                xr = xp.tile([128, CT, N], F32, tag="xraw")
                xraws.append(xr)
            for t in range(CT):
                eng = nc.sync if t < 2 else nc.scalar
                eng.dma_start(out=xraws[0][:, t, :], in_=x_ext[0, t * 128:(t + 1) * 128, :])

            # ---------------- per-core constants ----------------
            wqkv = const.tile([128, CT, 3 * C], BF16, tag="wqkv")
            wo = const.tile([128, CT, C], BF16, tag="wo")
            g1 = const.tile([128, CT], F32, tag="g1")
            g1q = const.tile([128, CT], F32, tag="g1q")
            ones128 = const.tile([128, 128], BF16, tag="ones128")
            ones1 = const.tile([128, 64], F32R, tag="ones1")
            kTp = const.tile([128, HEADS, 1028], BF16, tag="kTp")
            vextA = const.tile([128, 8, VW], BF16, tag="vextA")
            vextB = const.tile([128, 8, VW], BF16, tag="vextB")
            vmem = const.tile([128, 2, VW], BF16, tag="vmem")
            vexts = [vextA, vextB]

            gsb = const.tile([128, CT], F32, tag="gsb")
            nc.sync.dma_start(out=gsb, in_=gammat_ext[:, :])
            nc.scalar.activation(out=g1, in_=gsb, func=AF.Copy, bias=1.0)
            nc.scalar.activation(out=g1q, in_=gsb, func=AF.Copy, bias=1.0, scale=1.0)
            nc.scalar.mul(out=g1q, in_=g1q, mul=DH ** -0.5)

            nc.vector.memset(ones128, 1.0)
            nc.vector.memset(ones1.bitcast(F32), 1.0)

            def weight_prep():
                nc.gpsimd.memset(kTp, 0.0)
                for t in range(CT):
                    ws = wstage.tile([128, 3 * C], F32, tag="ws")
                    nc.sync.dma_start(out=ws, in_=wqkvt_ext[t * 128:(t + 1) * 128, :])
                    nc.vector.tensor_scalar_mul(
                        out=wqkv[:, t, 0:C], in0=ws[:, 0:C], scalar1=g1q[:, t:t + 1])
                    nc.vector.tensor_scalar_mul(
                        out=wqkv[:, t, C:3 * C], in0=ws[:, C:3 * C], scalar1=g1[:, t:t + 1])
                for t in range(CT):
                    ws = wstage.tile([128, 3 * C], F32, tag="ws")
                    nc.sync.dma_start(out=ws[:, 0:C], in_=wot_ext[t * 128:(t + 1) * 128, :])
                    nc.vector.tensor_copy(out=wo[:, t, :], in_=ws[:, 0:C])
                # mem_kv constants
                ws = wstage.tile([128, 3 * C], F32, tag="ws")
                nc.sync.dma_start(out=ws[:, 0:HEADS * NMEM],
                                  in_=memk_ext[:, :, :].rearrange("p h c -> p (h c)"))
                nc.sync.dma_start(out=ws[:, HEADS * NMEM:HEADS * NMEM + 2 * VW],
                                  in_=memv_ext[:, :, :].rearrange("p g c -> p (g c)"))
                nc.vector.tensor_copy(
                    out=kTp[:, :, 1024:1028],
                    in_=ws[:, 0:HEADS * NMEM].rearrange("p (h c) -> p h c", c=NMEM))
                nc.vector.tensor_copy(
                    out=vmem,
                    in_=ws[:, HEADS * NMEM:HEADS * NMEM + 2 * VW].rearrange("p (g c) -> p g c", c=VW))
                for v in vexts:
                    oc = v[:, :, :].rearrange("p j (h c) -> p j h c", c=DH + 1)[:, :, :, DH:DH + 1]
                    nc.gpsimd.memset(oc, 1.0)

            # ---------------- pipeline stages ----------------
            def norm(bb):
                """x -> xn (fp32r, per-pixel normalized)."""
                xraw = xraws[bb]
                xsq = data.tile([128, CT, N], BF16, tag="xsq")
                for t in range(CT):
                    nc.vector.tensor_mul(out=xsq[:, t, :], in0=xraw[:, t, :], in1=xraw[:, t, :])
                ss = sim_ps.tile([128, N], F32, tag="sim")
                for h2 in range(2):
                    for t in range(CT):
                        nc.tensor.matmul(ss[:, h2 * 512:(h2 + 1) * 512], ones128,
                                         xsq[:, t, h2 * 512:(h2 + 1) * 512],
                                         start=(t == 0), stop=(t == CT - 1))
                sroot = data.tile([128, N], F32, tag="sroot")
                nc.scalar.activation(out=sroot, in_=ss, func=AF.Sqrt, scale=1.0 / C)
                snorm = data.tile([128, N], F32, tag="snorm")
                nc.vector.reciprocal_approx_fast(out=snorm, in_=sroot)
                xn = data.tile([128, CT, N], BF16, tag="xn" + str(bb))
                for t in range(CT):
                    nc.vector.tensor_mul(out=xn[:, t, :], in0=xraw[:, t, :], in1=snorm)
                return xn

            def qkproj(xn, qT, mcs):
                """o-chunks mcs of the q/k projection; k goes into kTp (padded)."""
                for mc in mcs:
                    for h2 in range(2):
                        ps = qkv_ps.tile([128, 512], F32, tag="q")
                        for t in range(CT):
                            nc.tensor.matmul(ps, wqkv[:, t, mc * 128:(mc + 1) * 128],
                                             xn[:, t, h2 * 512:(h2 + 1) * 512],
                                             start=(t == 0), stop=(t == CT - 1))
                        if mc < 4:
                            nc.vector.tensor_copy(out=qT[:, mc, h2 * 512:(h2 + 1) * 512], in_=ps)
                        else:
                            h0, h1 = 2 * (mc - 4), 2 * (mc - 4) + 1
                            nc.vector.tensor_copy(
                                out=kTp[0:64, h0, h2 * 512:(h2 + 1) * 512], in_=ps[0:64, :])
                            nc.vector.tensor_copy(
                                out=kTp[64:128, h1, h2 * 512:(h2 + 1) * 512], in_=ps[64:128, :])

            def vproj(xn, vext, ics):
                for ic in ics:
                    ps = qkv_ps.tile([128, 512], F32, tag="q")
                    for t in range(CT):
                        nc.tensor.matmul(ps, xn[:, t, ic * 128:(ic + 1) * 128],
                                         wqkv[:, t, 2 * C:3 * C],
                                         start=(t == 0), stop=(t == CT - 1))
                    ps_h = ps[:, :].rearrange("p (h c) -> p h c", c=DH)
                    vdst = vext[:, ic, :].rearrange("p (h c) -> p h c", c=DH + 1)[:, :, 0:DH]
                    nc.vector.tensor_copy(out=vdst, in_=ps_h)

            def head_attn(h, qT, vext, attn, pmem):
                av0 = av_ps.tile([65, 512], F32, tag="av")
                av1 = av_ps.tile([65, 512], F32, tag="av")
                avt = (av0, av1)
                for jc in range(8):
                    st = sim_ps.tile([128, N], F32, tag="sim")
                    for h2 in range(2):
                        nc.tensor.matmul(st[:, h2 * 512:(h2 + 1) * 512],
                                         kTp[:, h, jc * 128:(jc + 1) * 128],
                                         qT[:, h // 2, h2 * 512:(h2 + 1) * 512],
                                         start=True, stop=True)
                    p = pp.tile([128, N], BF16, tag="p")
                    nc.scalar.activation(out=p, in_=st, func=AF.Exp)
                    for h2 in range(2):
                        nc.tensor.matmul(avt[h2], vext[:, jc, h * (DH + 1):(h + 1) * (DH + 1)],
                                         p[:, h2 * 512:(h2 + 1) * 512],
                                         start=(jc == 0), stop=False)
                # mem_kv contribution from the shared per-4-head exp tiles
                g, r0 = h // 4, 32 * (h % 4)
                for h2 in range(2):
                    nc.tensor.matmul(avt[h2],
                                     vmem[r0:r0 + NMEM, g, (h % 4) * (DH + 1):(h % 4 + 1) * (DH + 1)],
                                     pmem[g][r0:r0 + NMEM, h2 * 512:(h2 + 1) * 512],
                                     start=False, stop=True, tile_position=(r0, 0))
                for h2 in range(2):
                    avb = avsp.tile([65, 512], F32R, tag="avs")
                    nc.vector.tensor_copy(out=avb, in_=avt[h2])
                    bc = av_ps.tile([64, 512], F32, tag="av")
                    nc.tensor.matmul(bc, ones1[64:65, :], avb[64:65, :], start=True, stop=True)
                    rcp = rp.tile([64, 512], F32, tag="rcp")
                    nc.vector.reciprocal_approx_fast(out=rcp, in_=bc)
                    nc.vector.tensor_mul(
                        out=attn[64 * (h % 2):64 * (h % 2) + 64, h // 2,
                                 h2 * 512:(h2 + 1) * 512],
                        in0=avb[0:64, :].bitcast(F32), in1=rcp)

            def proj(attn, bb):
                for mc in range(CT):
                    for h2 in range(2):
                        ps = qkv_ps.tile([128, 512], F32, tag="q")
                        for t in range(CT):
                            nc.tensor.matmul(ps, wo[:, t, mc * 128:(mc + 1) * 128],
                                             attn[:, t, h2 * 512:(h2 + 1) * 512],
                                             start=(t == 0), stop=(t == CT - 1))
                        ob = obp.tile([128, 512], F32, tag="ob")
                        nc.vector.tensor_copy(out=ob, in_=ps)
                        nc.sync.dma_start(
                            out=out_ext[bb, mc * 128:(mc + 1) * 128, h2 * 512:(h2 + 1) * 512],
                            in_=ob)

            def mem_sims(qT):
                pms = []
                for g in range(2):
                    st = sim_ps.tile([128, N], F32, tag="sim")
                    for h4 in range(4):
                        h = 4 * g + h4
                        for h2 in range(2):
                            nc.tensor.matmul(st[32 * h4:32 * h4 + NMEM, h2 * 512:(h2 + 1) * 512],
                                             kTp[:, h, 1024:1028],
                                             qT[:, h // 2, h2 * 512:(h2 + 1) * 512],
                                             start=True, stop=True, tile_position=(0, 32 * h4))
                    pmt = pm.tile([128, N], BF16, tag="pm")
                    nc.scalar.activation(out=pmt, in_=st, func=AF.Exp)
                    pms.append(pmt)
                return pms

            # ---------------- interleaved schedule ----------------
            xn0 = norm(0)
            weight_prep()
            for t in range(CT):
                nc.sync.dma_start(out=xraws[1][:, t, :], in_=x_ext[1, t * 128:(t + 1) * 128, :])
            qT0 = qp.tile([128, CT, N], BF16, tag="qT")
            qkproj(xn0, qT0, range(8))
            vproj(xn0, vexts[0], range(8))
            xn1 = norm(1)

            pmem0 = mem_sims(qT0)
            qT1 = qp.tile([128, CT, N], BF16, tag="qT")
            attn0 = data.tile([128, CT, N], BF16, tag="attn")
            for h in range(HEADS):
                head_attn(h, qT0, vexts[0], attn0, pmem0)
                # batch 1 projections fill the exp-bound bubbles; k chunks are
                # written into kTp right after batch 0 finishes reading them.
                qkproj(xn1, qT1, [h // 2] if h % 2 == 0 else [4 + (h - 1) // 2])
                vproj(xn1, vexts[1], [h])
            proj(attn0, 0)

            pmem1 = mem_sims(qT1)
            attn1 = data.tile([128, CT, N], BF16, tag="attn")
            for h in range(HEADS):
                head_attn(h, qT1, vexts[1], attn1, pmem1)
            proj(attn1, 1)
    nc.compile()
    return nc


_NC_CACHE = []


def kernel(x, gamma, mem_kv, w_qkv, w_out, _trace=False):
    x = np.asarray(x, dtype=np.float32)
    gamma = np.asarray(gamma, dtype=np.float32)
    mem_kv = np.asarray(mem_kv, dtype=np.float32)
    w_qkv = np.asarray(w_qkv, dtype=np.float32)
    w_out = np.asarray(w_out, dtype=np.float32)

    b, c, hh, ww = x.shape
    n = hh * ww
    xs = x.reshape(b, c, n)

    wqkvt = np.ascontiguousarray(w_qkv.T)          # [c, 3c]
    wot = np.ascontiguousarray(w_out.T)            # [c, c]
    gammat = np.ascontiguousarray(gamma.reshape(CT, 128).T)  # [128, CT]

    memk = np.zeros((128, HEADS, NMEM), np.float32)
    memv = np.zeros((128, 2, VW), np.float32)
    for h in range(HEADS):
        r0 = 64 * (h % 2)
        memk[r0:r0 + DH, h, 0:NMEM] = mem_kv[0, h].T      # [dh, nmem]
        g, r1, c0 = h // 4, 32 * (h % 4), (h % 4) * (DH + 1)
        memv[r1:r1 + NMEM, g, c0:c0 + DH] = mem_kv[1, h]
        memv[r1:r1 + NMEM, g, c0 + DH] = 1.0

    if not _NC_CACHE:
        _NC_CACHE.append(_build())
    nc = _NC_CACHE[0]

    in_maps = []
    for core in range(NCORES):
        in_maps.append({
            "x": np.ascontiguousarray(xs[core * PB:(core + 1) * PB]),
            "wqkvt": wqkvt,
            "wot": wot,
            "gammat": gammat,
            "memk": memk,
            "memv": memv,
        })
    res = run_bass_kernel_spmd(nc, in_maps, core_ids=list(range(NCORES)), trace=_trace)
    out = np.concatenate([res.results[core]["out"] for core in range(NCORES)], axis=0)
    kernel.last_result = res
    return out.reshape(b, c, hh, ww)


# revision 30
# speedup vs baseline: 1.1209x; 1.0020x over previous
"""Trainium2 Bass kernel for nn_Attention_7945689497706.

Distribution: data-parallel over batch, 2 batch elements per core, weights
replicated, no collectives.

Per-core layout:
  - RMSNorm via ones-matmul partition reduction, gamma folded into weights.
  - q^T,k^T in [o, n] fp32r; v in [n, o] feeding a bf16 [v|1] (j, 65) tile.
  - Attention transposed (j on psum partitions): sim_T = kTpad^T qT with K
    zero-padded to 128; exp on ACT at [128,1024] grain; av lhsT = vext so the
    ones column accumulates softmax denominators; normalization = K=1 matmul
    broadcast + DVE fast-reciprocal + multiply.
  - mem_kv + padding in a 9th j-chunk (zero k-cols / zero v-rows make the
    padded lanes contribute nothing).
  - The two batch elements are software-pipelined: batch 1's norm/projections
    are emitted inside batch 0's attention loop (per-head kTp handoff) so the
    PE fills the ACT-bound exp bubbles.
"""

import numpy as np

import concourse.bass as bass
import concourse.mybir as mybir
import concourse.tile as tile
from concourse import bacc
from concourse.bass_utils import run_bass_kernel_spmd

F32 = mybir.dt.float32
F32R = mybir.dt.float32r
BF16 = mybir.dt.bfloat16
AF = mybir.ActivationFunctionType

NCORES = 8
B = 16
C = 512
N = 1024          # pixels = 32*32
HEADS = 8
DH = 64
NMEM = 4
PB = B // NCORES  # batch elements per core
CT = C // 128     # channel partition-tiles
JC = 9            # j chunks: 8 pixel chunks + 1 (mem + zero pad)
VW = HEADS * (DH + 1)  # vext width: per head [v | ones] = 65


def _build():
    nc = bacc.Bacc()
    x_ext = nc.declare_dram_parameter("x", [PB, C, N], F32, isOutput=False)
    wqkvt_ext = nc.declare_dram_parameter("wqkvt", [C, 3 * C], F32, isOutput=False)
    wot_ext = nc.declare_dram_parameter("wot", [C, C], F32, isOutput=False)
    gammat_ext = nc.declare_dram_parameter("gammat", [128, CT], F32, isOutput=False)
    memk_ext = nc.declare_dram_parameter("memk", [128, HEADS, NMEM], F32, isOutput=False)
    memv_ext = nc.declare_dram_parameter("memv", [128, 2, VW], F32, isOutput=False)
    out_ext = nc.declare_dram_parameter("out", [PB, C, N], F32, isOutput=True)

    with tile.TileContext(nc) as tc:
        with (
            tc.tile_pool(name="const", bufs=1) as const,
            tc.tile_pool(name="wstage", bufs=2) as wstage,
            tc.tile_pool(name="xp", bufs=2) as xp,
            tc.tile_pool(name="data", bufs=1) as data,
            tc.tile_pool(name="qp", bufs=2) as qp,
            tc.tile_pool(name="pp", bufs=6) as pp,
            tc.tile_pool(name="pm", bufs=4) as pm,
            tc.tile_pool(name="avs", bufs=3) as avsp,
            tc.tile_pool(name="rp", bufs=3) as rp,
            tc.tile_pool(name="ob", bufs=2) as obp,
            tc.tile_pool(name="qkv_ps", bufs=2, space="PSUM") as qkv_ps,
            tc.tile_pool(name="sim_ps", bufs=2, space="PSUM") as sim_ps,
            tc.tile_pool(name="av_ps", bufs=2, space="PSUM") as av_ps,
        ):
            # ------------ batch-0 x load first (weights stream behind it) -------
            xraws = []
            for bb in range(PB):
                xr = xp.tile([128, CT, N], F32, tag="xraw")
                xraws.append(xr)
            for t in range(CT):
                eng = nc.sync if t < 2 else nc.scalar
                eng.dma_start(out=xraws[0][:, t, :], in_=x_ext[0, t * 128:(t + 1) * 128, :])

            # ---------------- per-core constants ----------------
            wqkv = const.tile([128, CT, 3 * C], BF16, tag="wqkv")
            wo = const.tile([128, CT, C], BF16, tag="wo")
            g1 = const.tile([128, CT], F32, tag="g1")
            g1q = const.tile([128, CT], F32, tag="g1q")
            ones128 = const.tile([128, 128], BF16, tag="ones128")
            ones1 = const.tile([128, 64], F32R, tag="ones1")
            kTp = const.tile([128, HEADS, 1028], BF16, tag="kTp")
            vextA = const.tile([128, 8, VW], BF16, tag="vextA")
            vextB = const.tile([128, 8, VW], BF16, tag="vextB")
            vmem = const.tile([128, 2, VW], BF16, tag="vmem")
            vexts = [vextA, vextB]

            gsb = const.tile([128, CT], F32, tag="gsb")
            nc.sync.dma_start(out=gsb, in_=gammat_ext[:, :])
            nc.scalar.activation(out=g1, in_=gsb, func=AF.Copy, bias=1.0)
            nc.scalar.activation(out=g1q, in_=gsb, func=AF.Copy, bias=1.0, scale=1.0)
            nc.scalar.mul(out=g1q, in_=g1q, mul=DH ** -0.5)

            nc.vector.memset(ones128, 1.0)
            nc.vector.memset(ones1.bitcast(F32), 1.0)

            def weight_prep():
                nc.gpsimd.memset(kTp, 0.0)
                for t in range(CT):
                    ws = wstage.tile([128, 3 * C], F32, tag="ws")
                    nc.sync.dma_start(out=ws, in_=wqkvt_ext[t * 128:(t + 1) * 128, :])
                    nc.vector.tensor_scalar_mul(
                        out=wqkv[:, t, 0:C], in0=ws[:, 0:C], scalar1=g1q[:, t:t + 1])
                    nc.vector.tensor_scalar_mul(
                        out=wqkv[:, t, C:3 * C], in0=ws[:, C:3 * C], scalar1=g1[:, t:t + 1])
                for t in range(CT):
                    ws = wstage.tile([128, 3 * C], F32, tag="ws")
                    nc.sync.dma_start(out=ws[:, 0:C], in_=wot_ext[t * 128:(t + 1) * 128, :])
                    nc.vector.tensor_copy(out=wo[:, t, :], in_=ws[:, 0:C])
                # mem_kv constants
                ws = wstage.tile([128, 3 * C], F32, tag="ws")
                nc.sync.dma_start(out=ws[:, 0:HEADS * NMEM],
                                  in_=memk_ext[:, :, :].rearrange("p h c -> p (h c)"))
                nc.sync.dma_start(out=ws[:, HEADS * NMEM:HEADS * NMEM + 2 * VW],
                                  in_=memv_ext[:, :, :].rearrange("p g c -> p (g c)"))
                nc.vector.tensor_copy(
                    out=kTp[:, :, 1024:1028],
                    in_=ws[:, 0:HEADS * NMEM].rearrange("p (h c) -> p h c", c=NMEM))
                nc.vector.tensor_copy(
                    out=vmem,
                    in_=ws[:, HEADS * NMEM:HEADS * NMEM + 2 * VW].rearrange("p (g c) -> p g c", c=VW))
                for v in vexts:
                    oc = v[:, :, :].rearrange("p j (h c) -> p j h c", c=DH + 1)[:, :, :, DH:DH + 1]
                    nc.gpsimd.memset(oc, 1.0)

            # ---------------- pipeline stages ----------------
            def norm(bb):
                """x -> xn (fp32r, per-pixel normalized)."""
                xraw = xraws[bb]
                xsq = data.tile([128, CT, N], BF16, tag="xsq")
                for t in range(CT):
                    nc.vector.tensor_mul(out=xsq[:, t, :], in0=xraw[:, t, :], in1=xraw[:, t, :])
                ss = sim_ps.tile([128, N], F32, tag="sim")
                for h2 in range(2):
                    for t in range(CT):
                        nc.tensor.matmul(ss[:, h2 * 512:(h2 + 1) * 512], ones128,
                                         xsq[:, t, h2 * 512:(h2 + 1) * 512],
                                         start=(t == 0), stop=(t == CT - 1))
                sroot = data.tile([128, N], F32, tag="sroot")
                nc.scalar.activation(out=sroot, in_=ss, func=AF.Sqrt, scale=1.0 / C)
                snorm = data.tile([128, N], F32, tag="snorm")
                nc.vector.reciprocal_approx_fast(out=snorm, in_=sroot)
                xn = data.tile([128, CT, N], BF16, tag="xn" + str(bb))
                for t in range(CT):
                    nc.vector.tensor_mul(out=xn[:, t, :], in0=xraw[:, t, :], in1=snorm)
                return xn

            def qkproj(xn, qT, mcs):
                """o-chunks mcs of the q/k projection; k goes into kTp (padded)."""
                for mc in mcs:
                    for h2 in range(2):
                        ps = qkv_ps.tile([128, 512], F32, tag="q")
                        for t in range(CT):
                            nc.tensor.matmul(ps, wqkv[:, t, mc * 128:(mc + 1) * 128],
                                             xn[:, t, h2 * 512:(h2 + 1) * 512],
                                             start=(t == 0), stop=(t == CT - 1))
                        if mc < 4:
                            nc.vector.tensor_copy(out=qT[:, mc, h2 * 512:(h2 + 1) * 512], in_=ps)
                        else:
                            h0, h1 = 2 * (mc - 4), 2 * (mc - 4) + 1
                            nc.vector.tensor_copy(
                                out=kTp[0:64, h0, h2 * 512:(h2 + 1) * 512], in_=ps[0:64, :])
                            nc.vector.tensor_copy(
                                out=kTp[64:128, h1, h2 * 512:(h2 + 1) * 512], in_=ps[64:128, :])

            def vproj(xn, vext, ics):
                for ic in ics:
                    ps = qkv_ps.tile([128, 512], F32, tag="q")
                    for t in range(CT):
                        nc.tensor.matmul(ps, xn[:, t, ic * 128:(ic + 1) * 128],
                                         wqkv[:, t, 2 * C:3 * C],
                                         start=(t == 0), stop=(t == CT - 1))
                    ps_h = ps[:, :].rearrange("p (h c) -> p h c", c=DH)
                    vdst = vext[:, ic, :].rearrange("p (h c) -> p h c", c=DH + 1)[:, :, 0:DH]
                    nc.vector.tensor_copy(out=vdst, in_=ps_h)

            def head_attn(h, qT, vext, attn, pmem):
                av0 = av_ps.tile([65, 512], F32, tag="av")
                av1 = av_ps.tile([65, 512], F32, tag="av")
                avt = (av0, av1)
                for jc in range(8):
                    st = sim_ps.tile([128, N], F32, tag="sim")
                    for h2 in range(2):
                        nc.tensor.matmul(st[:, h2 * 512:(h2 + 1) * 512],
                                         kTp[:, h, jc * 128:(jc + 1) * 128],
                                         qT[:, h // 2, h2 * 512:(h2 + 1) * 512],
                                         start=True, stop=True)
                    p = pp.tile([128, N], BF16, tag="p")
                    nc.scalar.activation(out=p, in_=st, func=AF.Exp)
                    for h2 in range(2):
                        nc.tensor.matmul(avt[h2], vext[:, jc, h * (DH + 1):(h + 1) * (DH + 1)],
                                         p[:, h2 * 512:(h2 + 1) * 512],
                                         start=(jc == 0), stop=False)
                # mem_kv contribution from the shared per-4-head exp tiles
                g, r0 = h // 4, 32 * (h % 4)
                for h2 in range(2):
                    nc.tensor.matmul(avt[h2],
                                     vmem[r0:r0 + NMEM, g, (h % 4) * (DH + 1):(h % 4 + 1) * (DH + 1)],
                                     pmem[g][r0:r0 + NMEM, h2 * 512:(h2 + 1) * 512],
                                     start=False, stop=True, tile_position=(r0, 0))
                for h2 in range(2):
                    avb = avsp.tile([65, 512], F32R, tag="avs")
                    nc.vector.tensor_copy(out=avb, in_=avt[h2])
                    bc = av_ps.tile([64, 512], F32, tag="av")
                    nc.tensor.matmul(bc, ones1[64:65, :], avb[64:65, :], start=True, stop=True)
                    rcp = rp.tile([64, 512], F32, tag="rcp")
                    nc.vector.reciprocal_approx_fast(out=rcp, in_=bc)
                    nc.vector.tensor_mul(
                        out=attn[64 * (h % 2):64 * (h % 2) + 64, h // 2,
                                 h2 * 512:(h2 + 1) * 512],
                        in0=avb[0:64, :].bitcast(F32), in1=rcp)

            def proj(attn, bb):
                for mc in range(CT):
                    for h2 in range(2):
                        ps = qkv_ps.tile([128, 512], F32, tag="q")
                        for t in range(CT):
                            nc.tensor.matmul(ps, wo[:, t, mc * 128:(mc + 1) * 128],
                                             attn[:, t, h2 * 512:(h2 + 1) * 512],
                                             start=(t == 0), stop=(t == CT - 1))
                        ob = obp.tile([128, 512], F32, tag="ob")
                        nc.vector.tensor_copy(out=ob, in_=ps)
                        nc.sync.dma_start(
                            out=out_ext[bb, mc * 128:(mc + 1) * 128, h2 * 512:(h2 + 1) * 512],
                            in_=ob)

            def mem_sims(qT):
                pms = []
                for g in range(2):
                    st = sim_ps.tile([128, N], F32, tag="sim")
                    for h4 in range(4):
                        h = 4 * g + h4
                        for h2 in range(2):
                            nc.tensor.matmul(st[32 * h4:32 * h4 + NMEM, h2 * 512:(h2 + 1) * 512],
                                             kTp[:, h, 1024:1028],
                                             qT[:, h // 2, h2 * 512:(h2 + 1) * 512],
                                             start=True, stop=True, tile_position=(0, 32 * h4))
                    pmt = pm.tile([128, N], BF16, tag="pm")
                    nc.scalar.activation(out=pmt, in_=st, func=AF.Exp)
                    pms.append(pmt)
                return pms

            # ---------------- interleaved schedule ----------------
            xn0 = norm(0)
            weight_prep()
            for t in range(CT):
                nc.sync.dma_start(out=xraws[1][:, t, :], in_=x_ext[1, t * 128:(t + 1) * 128, :])
            qT0 = qp.tile([128, CT, N], BF16, tag="qT")
            qkproj(xn0, qT0, range(8))
            vproj(xn0, vexts[0], range(8))
            xn1 = norm(1)

            pmem0 = mem_sims(qT0)
            qT1 = qp.tile([128, CT, N], BF16, tag="qT")
            attn0 = data.tile([128, CT, N], BF16, tag="attn")
            for h in range(HEADS):
                # batch 1 projections fill the exp-bound bubbles; k chunks are
                # written into kTp right after batch 0 finishes reading them.
                if h % 2 == 0:
                    qkproj(xn1, qT1, [h // 2])
                vproj(xn1, vexts[1], [h])
                head_attn(h, qT0, vexts[0], attn0, pmem0)
                if h % 2 == 1:
                    qkproj(xn1, qT1, [4 + (h - 1) // 2])
            proj(attn0, 0)

            pmem1 = mem_sims(qT1)
            attn1 = data.tile([128, CT, N], BF16, tag="attn")
            for h in range(HEADS):
                head_attn(h, qT1, vexts[1], attn1, pmem1)
            proj(attn1, 1)
    nc.compile()
    return nc


_NC_CACHE = []


def kernel(x, gamma, mem_kv, w_qkv, w_out, _trace=False):
    x = np.asarray(x, dtype=np.float32)
    gamma = np.asarray(gamma, dtype=np.float32)
    mem_kv = np.asarray(mem_kv, dtype=np.float32)
    w_qkv = np.asarray(w_qkv, dtype=np.float32)
    w_out = np.asarray(w_out, dtype=np.float32)

    b, c, hh, ww = x.shape
    n = hh * ww
    xs = x.reshape(b, c, n)

    wqkvt = np.ascontiguousarray(w_qkv.T)          # [c, 3c]
    wot = np.ascontiguousarray(w_out.T)            # [c, c]
    gammat = np.ascontiguousarray(gamma.reshape(CT, 128).T)  # [128, CT]

    memk = np.zeros((128, HEADS, NMEM), np.float32)
    memv = np.zeros((128, 2, VW), np.float32)
    for h in range(HEADS):
        r0 = 64 * (h % 2)
        memk[r0:r0 + DH, h, 0:NMEM] = mem_kv[0, h].T      # [dh, nmem]
        g, r1, c0 = h // 4, 32 * (h % 4), (h % 4) * (DH + 1)
        memv[r1:r1 + NMEM, g, c0:c0 + DH] = mem_kv[1, h]
        memv[r1:r1 + NMEM, g, c0 + DH] = 1.0

    if not _NC_CACHE:
        _NC_CACHE.append(_build())
    nc = _NC_CACHE[0]

    in_maps = []
    for core in range(NCORES):
        in_maps.append({
            "x": np.ascontiguousarray(xs[core * PB:(core + 1) * PB]),
            "wqkvt": wqkvt,
            "wot": wot,
            "gammat": gammat,
            "memk": memk,
            "memv": memv,
        })
    res = run_bass_kernel_spmd(nc, in_maps, core_ids=list(range(NCORES)), trace=_trace)
    out = np.concatenate([res.results[core]["out"] for core in range(NCORES)], axis=0)
    kernel.last_result = res
    return out.reshape(b, c, hh, ww)


# revision 31
# speedup vs baseline: 1.1436x; 1.0203x over previous
"""Trainium2 Bass kernel for nn_Attention_7945689497706.

Distribution: data-parallel over batch, 2 batch elements per core, weights
replicated, no collectives.

Per-core layout:
  - RMSNorm via ones-matmul partition reduction, gamma folded into weights.
  - q^T,k^T in [o, n] fp32r; v in [n, o] feeding a bf16 [v|1] (j, 65) tile.
  - Attention transposed (j on psum partitions): sim_T = kTpad^T qT with K
    zero-padded to 128; exp on ACT at [128,1024] grain; av lhsT = vext so the
    ones column accumulates softmax denominators; normalization = K=1 matmul
    broadcast + DVE fast-reciprocal + multiply.
  - mem_kv + padding in a 9th j-chunk (zero k-cols / zero v-rows make the
    padded lanes contribute nothing).
  - The two batch elements are software-pipelined: batch 1's norm/projections
    are emitted inside batch 0's attention loop (per-head kTp handoff) so the
    PE fills the ACT-bound exp bubbles.
"""

import numpy as np

import concourse.bass as bass
import concourse.mybir as mybir
import concourse.tile as tile
from concourse import bacc
from concourse.bass_utils import run_bass_kernel_spmd

F32 = mybir.dt.float32
F32R = mybir.dt.float32r
BF16 = mybir.dt.bfloat16
AF = mybir.ActivationFunctionType

NCORES = 8
B = 16
C = 512
N = 1024          # pixels = 32*32
HEADS = 8
DH = 64
NMEM = 4
PB = B // NCORES  # batch elements per core
CT = C // 128     # channel partition-tiles
JC = 9            # j chunks: 8 pixel chunks + 1 (mem + zero pad)
VW = HEADS * (DH + 1)  # vext width: per head [v | ones] = 65


def _build():
    nc = bacc.Bacc()
    x_ext = nc.declare_dram_parameter("x", [PB, C, N], F32, isOutput=False)
    wqkvt_ext = nc.declare_dram_parameter("wqkvt", [C, 3 * C], F32, isOutput=False)
    wot_ext = nc.declare_dram_parameter("wot", [C, C], F32, isOutput=False)
    gammat_ext = nc.declare_dram_parameter("gammat", [128, CT], F32, isOutput=False)
    memk_ext = nc.declare_dram_parameter("memk", [128, HEADS, NMEM], F32, isOutput=False)
    memv_ext = nc.declare_dram_parameter("memv", [128, 2, VW], F32, isOutput=False)
    out_ext = nc.declare_dram_parameter("out", [PB, C, N], F32, isOutput=True)

    with tile.TileContext(nc) as tc:
        with (
            tc.tile_pool(name="const", bufs=1) as const,
            tc.tile_pool(name="wstage", bufs=2) as wstage,
            tc.tile_pool(name="xp", bufs=2) as xp,
            tc.tile_pool(name="data", bufs=1) as data,
            tc.tile_pool(name="qp", bufs=2) as qp,
            tc.tile_pool(name="pp", bufs=6) as pp,
            tc.tile_pool(name="pm", bufs=4) as pm,
            tc.tile_pool(name="avs", bufs=4) as avsp,
            tc.tile_pool(name="rp", bufs=4) as rp,
            tc.tile_pool(name="ob", bufs=4) as obp,
            tc.tile_pool(name="qkv_ps", bufs=2, space="PSUM") as qkv_ps,
            tc.tile_pool(name="sim_ps", bufs=2, space="PSUM") as sim_ps,
            tc.tile_pool(name="av_ps", bufs=2, space="PSUM") as av_ps,
        ):
            # ------------ batch-0 x load first (weights stream behind it) -------
            xraws = []
            for bb in range(PB):
                xr = xp.tile([128, CT, N], F32, tag="xraw")
                xraws.append(xr)
            for t in range(CT):
                eng = nc.sync if t < 2 else nc.scalar
                eng.dma_start(out=xraws[0][:, t, :], in_=x_ext[0, t * 128:(t + 1) * 128, :])

            # ---------------- per-core constants ----------------
            wqkv = const.tile([128, CT, 3 * C], BF16, tag="wqkv")
            wo = const.tile([128, CT, C], BF16, tag="wo")
            g1 = const.tile([128, CT], F32, tag="g1")
            g1q = const.tile([128, CT], F32, tag="g1q")
            ones128 = const.tile([128, 128], BF16, tag="ones128")
            ones1 = const.tile([128, 64], F32R, tag="ones1")
            kTp = const.tile([128, HEADS, 1028], BF16, tag="kTp")
            vextA = const.tile([128, 8, VW], BF16, tag="vextA")
            vextB = const.tile([128, 8, VW], BF16, tag="vextB")
            vmem = const.tile([128, 2, VW], BF16, tag="vmem")
            vexts = [vextA, vextB]

            gsb = const.tile([128, CT], F32, tag="gsb")
            nc.sync.dma_start(out=gsb, in_=gammat_ext[:, :])
            nc.scalar.activation(out=g1, in_=gsb, func=AF.Copy, bias=1.0)
            nc.scalar.activation(out=g1q, in_=gsb, func=AF.Copy, bias=1.0, scale=1.0)
            nc.scalar.mul(out=g1q, in_=g1q, mul=DH ** -0.5)

            nc.vector.memset(ones128, 1.0)
            nc.vector.memset(ones1.bitcast(F32), 1.0)

            def weight_prep():
                nc.gpsimd.memset(kTp, 0.0)
                for t in range(CT):
                    ws = wstage.tile([128, 3 * C], F32, tag="ws")
                    nc.sync.dma_start(out=ws, in_=wqkvt_ext[t * 128:(t + 1) * 128, :])
                    nc.vector.tensor_scalar_mul(
                        out=wqkv[:, t, 0:C], in0=ws[:, 0:C], scalar1=g1q[:, t:t + 1])
                    nc.vector.tensor_scalar_mul(
                        out=wqkv[:, t, C:3 * C], in0=ws[:, C:3 * C], scalar1=g1[:, t:t + 1])
                for t in range(CT):
                    ws = wstage.tile([128, 3 * C], F32, tag="ws")
                    nc.sync.dma_start(out=ws[:, 0:C], in_=wot_ext[t * 128:(t + 1) * 128, :])
                    nc.vector.tensor_copy(out=wo[:, t, :], in_=ws[:, 0:C])
                # mem_kv constants
                ws = wstage.tile([128, 3 * C], F32, tag="ws")
                nc.sync.dma_start(out=ws[:, 0:HEADS * NMEM],
                                  in_=memk_ext[:, :, :].rearrange("p h c -> p (h c)"))
                nc.sync.dma_start(out=ws[:, HEADS * NMEM:HEADS * NMEM + 2 * VW],
                                  in_=memv_ext[:, :, :].rearrange("p g c -> p (g c)"))
                nc.vector.tensor_copy(
                    out=kTp[:, :, 1024:1028],
                    in_=ws[:, 0:HEADS * NMEM].rearrange("p (h c) -> p h c", c=NMEM))
                nc.vector.tensor_copy(
                    out=vmem,
                    in_=ws[:, HEADS * NMEM:HEADS * NMEM + 2 * VW].rearrange("p (g c) -> p g c", c=VW))
                for v in vexts:
                    oc = v[:, :, :].rearrange("p j (h c) -> p j h c", c=DH + 1)[:, :, :, DH:DH + 1]
                    nc.gpsimd.memset(oc, 1.0)

            # ---------------- pipeline stages ----------------
            def norm(bb):
                """x -> xn (fp32r, per-pixel normalized)."""
                xraw = xraws[bb]
                xsq = data.tile([128, CT, N], BF16, tag="xsq")
                for t in range(CT):
                    nc.vector.tensor_mul(out=xsq[:, t, :], in0=xraw[:, t, :], in1=xraw[:, t, :])
                ss = sim_ps.tile([128, N], F32, tag="sim")
                for h2 in range(2):
                    for t in range(CT):
                        nc.tensor.matmul(ss[:, h2 * 512:(h2 + 1) * 512], ones128,
                                         xsq[:, t, h2 * 512:(h2 + 1) * 512],
                                         start=(t == 0), stop=(t == CT - 1))
                sroot = data.tile([128, N], F32, tag="sroot")
                nc.scalar.activation(out=sroot, in_=ss, func=AF.Sqrt, scale=1.0 / C)
                snorm = data.tile([128, N], F32, tag="snorm")
                nc.vector.reciprocal_approx_fast(out=snorm, in_=sroot)
                xn = data.tile([128, CT, N], BF16, tag="xn" + str(bb))
                for t in range(CT):
                    nc.vector.tensor_mul(out=xn[:, t, :], in0=xraw[:, t, :], in1=snorm)
                return xn

            def qkproj(xn, qT, mcs):
                """o-chunks mcs of the q/k projection; k goes into kTp (padded)."""
                for mc in mcs:
                    for h2 in range(2):
                        ps = qkv_ps.tile([128, 512], F32, tag="q")
                        for t in range(CT):
                            nc.tensor.matmul(ps, wqkv[:, t, mc * 128:(mc + 1) * 128],
                                             xn[:, t, h2 * 512:(h2 + 1) * 512],
                                             start=(t == 0), stop=(t == CT - 1))
                        if mc < 4:
                            nc.vector.tensor_copy(out=qT[:, mc, h2 * 512:(h2 + 1) * 512], in_=ps)
                        else:
                            h0, h1 = 2 * (mc - 4), 2 * (mc - 4) + 1
                            nc.vector.tensor_copy(
                                out=kTp[0:64, h0, h2 * 512:(h2 + 1) * 512], in_=ps[0:64, :])
                            nc.vector.tensor_copy(
                                out=kTp[64:128, h1, h2 * 512:(h2 + 1) * 512], in_=ps[64:128, :])

            def vproj(xn, vext, ics):
                for ic in ics:
                    ps = qkv_ps.tile([128, 512], F32, tag="q")
                    for t in range(CT):
                        nc.tensor.matmul(ps, xn[:, t, ic * 128:(ic + 1) * 128],
                                         wqkv[:, t, 2 * C:3 * C],
                                         start=(t == 0), stop=(t == CT - 1))
                    ps_h = ps[:, :].rearrange("p (h c) -> p h c", c=DH)
                    vdst = vext[:, ic, :].rearrange("p (h c) -> p h c", c=DH + 1)[:, :, 0:DH]
                    nc.vector.tensor_copy(out=vdst, in_=ps_h)

            def head_attn(h, qT, vext, attn, pmem):
                av0 = av_ps.tile([65, 512], F32, tag="av")
                av1 = av_ps.tile([65, 512], F32, tag="av")
                avt = (av0, av1)
                for jc in range(8):
                    st = sim_ps.tile([128, N], F32, tag="sim")
                    for h2 in range(2):
                        nc.tensor.matmul(st[:, h2 * 512:(h2 + 1) * 512],
                                         kTp[:, h, jc * 128:(jc + 1) * 128],
                                         qT[:, h // 2, h2 * 512:(h2 + 1) * 512],
                                         start=True, stop=True)
                    p = pp.tile([128, N], BF16, tag="p")
                    nc.scalar.activation(out=p, in_=st, func=AF.Exp)
                    for h2 in range(2):
                        nc.tensor.matmul(avt[h2], vext[:, jc, h * (DH + 1):(h + 1) * (DH + 1)],
                                         p[:, h2 * 512:(h2 + 1) * 512],
                                         start=(jc == 0), stop=False)
                # mem_kv contribution from the shared per-4-head exp tiles
                g, r0 = h // 4, 32 * (h % 4)
                for h2 in range(2):
                    nc.tensor.matmul(avt[h2],
                                     vmem[r0:r0 + NMEM, g, (h % 4) * (DH + 1):(h % 4 + 1) * (DH + 1)],
                                     pmem[g][r0:r0 + NMEM, h2 * 512:(h2 + 1) * 512],
                                     start=False, stop=True, tile_position=(r0, 0))
                for h2 in range(2):
                    avb = avsp.tile([65, 512], F32R, tag="avs")
                    nc.vector.tensor_copy(out=avb, in_=avt[h2])
                    bc = av_ps.tile([64, 512], F32, tag="av")
                    nc.tensor.matmul(bc, ones1[64:65, :], avb[64:65, :], start=True, stop=True)
                    rcp = rp.tile([64, 512], F32, tag="rcp")
                    nc.vector.reciprocal_approx_fast(out=rcp, in_=bc)
                    nc.vector.tensor_mul(
                        out=attn[64 * (h % 2):64 * (h % 2) + 64, h // 2,
                                 h2 * 512:(h2 + 1) * 512],
                        in0=avb[0:64, :].bitcast(F32), in1=rcp)

            def proj(attn, bb):
                for mc in range(CT):
                    for h2 in range(2):
                        ps = qkv_ps.tile([128, 512], F32, tag="q")
                        for t in range(CT):
                            nc.tensor.matmul(ps, wo[:, t, mc * 128:(mc + 1) * 128],
                                             attn[:, t, h2 * 512:(h2 + 1) * 512],
                                             start=(t == 0), stop=(t == CT - 1))
                        ob = obp.tile([128, 512], F32, tag="ob")
                        nc.vector.tensor_copy(out=ob, in_=ps)
                        nc.sync.dma_start(
                            out=out_ext[bb, mc * 128:(mc + 1) * 128, h2 * 512:(h2 + 1) * 512],
                            in_=ob)

            def mem_sims(qT):
                pms = []
                for g in range(2):
                    st = sim_ps.tile([128, N], F32, tag="sim")
                    for h4 in range(4):
                        h = 4 * g + h4
                        for h2 in range(2):
                            nc.tensor.matmul(st[32 * h4:32 * h4 + NMEM, h2 * 512:(h2 + 1) * 512],
                                             kTp[:, h, 1024:1028],
                                             qT[:, h // 2, h2 * 512:(h2 + 1) * 512],
                                             start=True, stop=True, tile_position=(0, 32 * h4))
                    pmt = pm.tile([128, N], BF16, tag="pm")
                    nc.scalar.activation(out=pmt, in_=st, func=AF.Exp)
                    pms.append(pmt)
                return pms

            # ---------------- interleaved schedule ----------------
            xn0 = norm(0)
            weight_prep()
            for t in range(CT):
                nc.sync.dma_start(out=xraws[1][:, t, :], in_=x_ext[1, t * 128:(t + 1) * 128, :])
            qT0 = qp.tile([128, CT, N], BF16, tag="qT")
            qkproj(xn0, qT0, range(8))
            vproj(xn0, vexts[0], range(8))
            xn1 = norm(1)

            pmem0 = mem_sims(qT0)
            qT1 = qp.tile([128, CT, N], BF16, tag="qT")
            attn0 = data.tile([128, CT, N], BF16, tag="attn")
            for h in range(HEADS):
                # batch 1 projections fill the exp-bound bubbles; k chunks are
                # written into kTp right after batch 0 finishes reading them.
                if h % 2 == 0:
                    qkproj(xn1, qT1, [h // 2])
                vproj(xn1, vexts[1], [h])
                head_attn(h, qT0, vexts[0], attn0, pmem0)
                if h % 2 == 1:
                    qkproj(xn1, qT1, [4 + (h - 1) // 2])
            proj(attn0, 0)

            pmem1 = mem_sims(qT1)
            attn1 = data.tile([128, CT, N], BF16, tag="attn")
            for h in range(HEADS):
                head_attn(h, qT1, vexts[1], attn1, pmem1)
            proj(attn1, 1)
    nc.compile()
    return nc


_NC_CACHE = []


def kernel(x, gamma, mem_kv, w_qkv, w_out, _trace=False):
    x = np.asarray(x, dtype=np.float32)
    gamma = np.asarray(gamma, dtype=np.float32)
    mem_kv = np.asarray(mem_kv, dtype=np.float32)
    w_qkv = np.asarray(w_qkv, dtype=np.float32)
    w_out = np.asarray(w_out, dtype=np.float32)

    b, c, hh, ww = x.shape
    n = hh * ww
    xs = x.reshape(b, c, n)

    wqkvt = np.ascontiguousarray(w_qkv.T)          # [c, 3c]
    wot = np.ascontiguousarray(w_out.T)            # [c, c]
    gammat = np.ascontiguousarray(gamma.reshape(CT, 128).T)  # [128, CT]

    memk = np.zeros((128, HEADS, NMEM), np.float32)
    memv = np.zeros((128, 2, VW), np.float32)
    for h in range(HEADS):
        r0 = 64 * (h % 2)
        memk[r0:r0 + DH, h, 0:NMEM] = mem_kv[0, h].T      # [dh, nmem]
        g, r1, c0 = h // 4, 32 * (h % 4), (h % 4) * (DH + 1)
        memv[r1:r1 + NMEM, g, c0:c0 + DH] = mem_kv[1, h]
        memv[r1:r1 + NMEM, g, c0 + DH] = 1.0

    if not _NC_CACHE:
        _NC_CACHE.append(_build())
    nc = _NC_CACHE[0]

    in_maps = []
    for core in range(NCORES):
        in_maps.append({
            "x": np.ascontiguousarray(xs[core * PB:(core + 1) * PB]),
            "wqkvt": wqkvt,
            "wot": wot,
            "gammat": gammat,
            "memk": memk,
            "memv": memv,
        })
    res = run_bass_kernel_spmd(nc, in_maps, core_ids=list(range(NCORES)), trace=_trace)
    out = np.concatenate([res.results[core]["out"] for core in range(NCORES)], axis=0)
    kernel.last_result = res
    return out.reshape(b, c, hh, ww)
